# revision 34
# baseline (speedup 1.0000x reference)
"""Multi-Head Latent Attention (MLA) forward on 8 Trainium2 NeuronCores.

Sharding: tensor-parallel over heads (16 heads -> 2 per core), with the
latent-kv encoding sharded by TOKENS:
  - each core computes the full 576-dim kvc for its own 512-token slice
    from a 2MB bf16 slice of x.T; the result is distributed through FIVE
    pipelined AllGathers (one per 128-column tile), and the kv
    up-projection consumes each latent tile as it arrives,
  - q projections run as fp8e4 DoubleRow matmuls (K=256 per pass, 2x PE
    throughput) on an fp8 copy of x.T; the fp8 weight scale S is undone
    for free by the softmax exp's scale argument (exp(st/S)),
  - the latent path stays bf16 through the up-projection; k (nope+rope)
    and q are then quantized to fp8 so each score tile is a single
    DoubleRow matmul: k-tile 0 = k_nope, k-tile 1 = k_rope zero-padded,
    with per-head zero masks selecting that head's rope rows,
  - causal attention in transposed-score layout; exp/mask/denominator/
    attention-v work only on the valid lower-triangular column ranges of
    diagonal tiles; the denominator accumulates on the Vector engine in
    f32r so neither the PE nor the slow GpSimd gates the chain,
  - per-batch AllToAll exchanges head outputs; wo loads mid-flight and the
    last AllToAll is drained with a single rearranged DMA.
Output slices are disjoint; the host just concatenates them.
"""
import sys

if "/opt/trn_rl_repo" not in sys.path:
    sys.path.insert(0, "/opt/trn_rl_repo")

import numpy as np
import ml_dtypes
import concourse.bacc as bacc
import concourse.mybir as mybir
from concourse import tile
from concourse.bass_utils import run_bass_kernel_spmd

H, NOPE, ROPE, VD, KVR, QKD = 16, 128, 64, 128, 512, 192
B, T, D = 2, 2048, 2048
NCORES, HPC, BLK = 8, 2, 512
KVC = KVR + ROPE  # 576 latent+rope columns
S = 128.0  # fp8 weight scale, undone inside the softmax exp
f32 = mybir.dt.float32
f32r = mybir.dt.float32r
bf16 = mybir.dt.bfloat16
fp8 = mybir.dt.float8e4
DR = mybir.MatmulPerfMode.DoubleRow
EXP = mybir.ActivationFunctionType.Exp
LN = mybir.ActivationFunctionType.Ln
SQUARE = mybir.ActivationFunctionType.Square


def _patch_act_tables():
    """Make the act-table-load pass serve Exp/Ln/Square from the one set that
    contains them all (natural_log_exp_and_others), so interleaved activations
    don't thrash table loads. Indices into act_info.json must be preserved, so
    the shadowing single-function sets are emptied in place, not removed."""
    import concourse.bacc as _bacc

    orig = _bacc.get_activation_tables
    if getattr(_bacc, "_mla_act_patch", False):
        return
    _bacc._mla_act_patch = True

    def patched(arch):
        d = dict(orig(arch))
        if "natural_log_exp_and_others" in d:
            for name in ("exp_and_others", "natural_log", "exp_and_friends"):
                if name in d:
                    d[name] = set()
        return d

    _bacc.get_activation_tables = patched


def build_program():
    _patch_act_tables()
    nc = bacc.Bacc("TRN2", target_bir_lowering=False, debug=False, num_devices=NCORES)
    xt8_d = nc.dram_tensor("xt8", [D, B * T], fp8, kind="ExternalInput")
    xs_d = nc.dram_tensor("xs", [D, BLK], bf16, kind="ExternalInput")
    # w1 pre-permuted on host: rows (kp, p), cols (mt, two, mw) so each
    # DoubleRow stationary slice [128, 2, 128] is contiguous (ISA requirement)
    w1_d = nc.dram_tensor("w1", [D // 2, 768], fp8, kind="ExternalInput")
    wa_d = nc.dram_tensor("wa", [D, KVC], bf16, kind="ExternalInput")
    wb_d = nc.dram_tensor("wb", [KVR, HPC * (NOPE + VD)], bf16, kind="ExternalInput")
    wo_d = nc.dram_tensor("wo", [H * VD, D], bf16, kind="ExternalInput")
    cos_d = nc.dram_tensor("cos", [128, T], bf16, kind="ExternalInput")
    sin_d = nc.dram_tensor("sin", [128, T], bf16, kind="ExternalInput")
    msk_d = nc.dram_tensor("msk", [128, 4 * BLK], bf16, kind="ExternalInput")
    out_d = nc.dram_tensor("out", [B, T // NCORES, D], bf16, kind="ExternalOutput")

    RG = [list(range(NCORES))]

    with tile.TileContext(nc) as tc:
        with (
            tc.tile_pool(name="dram", bufs=1, space="DRAM") as dram,
            tc.tile_pool(name="const", bufs=1) as const,
            tc.tile_pool(name="wpool", bufs=1) as wpool,
            tc.tile_pool(name="kvpool", bufs=1) as kvpool,
            tc.tile_pool(name="work", bufs=1) as work,
            tc.tile_pool(name="wop", bufs=1) as wop,
            tc.tile_pool(name="ps", bufs=1, space="PSUM") as ps,
        ):
            y_in = [
                dram.tile([NCORES, HPC * VD, 256], bf16, name=f"y_in{b}")
                for b in range(B)
            ]
            y_out = [
                dram.tile([NCORES, HPC * VD, 256], bf16, name=f"y_out{b}")
                for b in range(B)
            ]
            # one AG per 128-col kvc tile (last is the 64-row rope tile)
            MTS = [128, 128, 128, 128, 64]
            ag_in = [
                dram.tile([MTS[mt], BLK], bf16, name=f"ag_in{mt}") for mt in range(5)
            ]
            ag_out = [
                dram.tile(
                    [NCORES * MTS[mt], BLK],
                    bf16,
                    name=f"ag_out{mt}",
                    addr_space="Shared",
                )
                for mt in range(5)
            ]

            agd_in = dram.tile([1, 64], bf16, name="agd_in")
            agd_out = dram.tile([8, 64], bf16, name="agd_out", addr_space="Shared")

            ones_f = const.tile([128, 1], f32, tag="ones_f")
            nc.gpsimd.memset(ones_f[:], 1.0)
            ones_r = const.tile([128, 1], f32r, tag="ones_r")
            nc.vector.tensor_copy(ones_r[:], ones_f[:])
            ones_b = const.tile([128, 1], bf16, tag="ones_b")
            nc.vector.tensor_copy(ones_b[:], ones_f[:])
            onesrow_f = const.tile([1, 128], f32, tag="onesrow_f")
            nc.gpsimd.memset(onesrow_f[:], 1.0)
            onesrow_b = const.tile([1, 128], bf16, tag="onesrow_b")
            nc.vector.tensor_copy(onesrow_b[:], onesrow_f[:])
            eps = const.tile([1, 1], f32, tag="eps")
            nc.gpsimd.memset(eps[:], 1e-6)

            dummy_sb = const.tile([1, 64], bf16, tag="dummy")
            nc.gpsimd.memset(dummy_sb[:], 0.0)
            nc.sync.dma_start(agd_in[:], dummy_sb[:])
            nc.gpsimd.collective_compute(
                "AllGather",
                mybir.AluOpType.bypass,
                replica_groups=RG,
                ins=[agd_in.opt()],
                outs=[agd_out.opt()],
            )

            # xs shares its SBUF slot with the later a2a staging tiles, and wa
            # with the big wo tile: both are dead before their partner loads.
            # Loads split into kc quarters so the kvc matmuls start ~4us in.
            xs_sb = wop.tile([128, 16, BLK], bf16, tag="a2a", bufs=1, name="xs")
            wa_sb = wpool.tile([128, 16, KVC], bf16, tag="wavo", bufs=1, name="wa")
            for g in range(4):
                kcs = slice(g * 4, g * 4 + 4)
                rows = slice(g * 512, (g + 1) * 512)
                nc.sync.dma_start(
                    xs_sb[:, kcs, :],
                    xs_d[rows, :].rearrange("(kc p) t -> p kc t", p=128),
                )
                nc.sync.dma_start(
                    wa_sb[:, kcs, :],
                    wa_d[rows, :].rearrange("(kc p) m -> p kc m", p=128),
                )
            w1_sb = wpool.tile([128, 8, 3, 2, 128], fp8, tag="w1")
            nc.sync.dma_start(
                w1_sb[:],
                w1_d[:].rearrange(
                    "(kp p) (mt two mw) -> p kp mt two mw", p=128, mt=3, two=2
                ),
            )
            wb_sb = wpool.tile([128, 4, 512], bf16, tag="wb")
            nc.sync.dma_start(wb_sb[:], wb_d[:].rearrange("(kc p) m -> p kc m", p=128))
            cs_sb = wpool.tile([128, T], bf16, tag="cs")
            nc.sync.dma_start(cs_sb[:], cos_d[:])
            sn_sb = wpool.tile([128, T], bf16, tag="sn")
            nc.sync.dma_start(sn_sb[:], sin_d[:])
            msk_sb = wpool.tile([128, 4 * BLK], bf16, tag="msk")

            def load_msk():
                nc.sync.dma_start(msk_sb[:], msk_d[:])

            wo_box = {}

            def load_wo():
                wo_box["wo"] = wpool.tile(
                    [128, 16, D], bf16, tag="wavo", bufs=1, name="wo"
                )
                nc.sync.dma_start(
                    wo_box["wo"][:], wo_d[:].rearrange("(kc p) m -> p kc m", p=128)
                )

            def kvc_block():
                """Full 576-col kvc for this core's 512-token slice; one AG
                per 128-col tile so downstream work pipelines with comms."""
                for mt in range(5):
                    m0 = mt * 128
                    mw = MTS[mt]
                    pc = ps.tile([128, BLK], f32, tag="proj", bufs=4, name="pc")
                    for kc in range(16):
                        nc.tensor.matmul(
                            pc[:mw, :],
                            wa_sb[:, kc, m0 : m0 + mw],
                            xs_sb[:, kc, :],
                            start=(kc == 0),
                            stop=(kc == 15),
                        )
                    kvcm = work.tile([128, BLK], bf16, tag="kvcm", bufs=2)
                    nc.vector.tensor_copy(kvcm[:mw, :], pc[:mw, :])
                    nc.sync.dma_start(ag_in[mt][:], kvcm[:mw, :])
                    nc.gpsimd.collective_compute(
                        "AllGather",
                        mybir.AluOpType.bypass,
                        replica_groups=RG,
                        ins=[ag_in[mt].opt()],
                        outs=[ag_out[mt].opt()],
                    )

            # per-batch kv staging in fp8 score layout: kn8[h] holds k-tile 0
            # (k_nope) and k-tile 1 (k_rope in that head's partition half,
            # zeros elsewhere); vnat stays bf16 for the attention-v matmul
            def alloc_kv(b):
                kn8 = [
                    kvpool.tile(
                        [128, 16, 2, 128], fp8, tag="kn8", bufs=2, name=f"kn8{b}_{h}"
                    )
                    for h in range(2)
                ]
                vnat = [
                    kvpool.tile(
                        [128, 16, VD], bf16, tag="vnat", bufs=2, name=f"vnat{b}_{h}"
                    )
                    for h in range(2)
                ]
                nc.gpsimd.memset(kn8[0][64:128, :, 1, :], 0.0)
                nc.gpsimd.memset(kn8[1][0:64, :, 1, :], 0.0)
                return kn8, vnat

            def stage_a1(b, qc):
                """fp8 x chunk load + DoubleRow q projection + q rope.

                qf8[h, 0] = S*q_nope(h); qf8[*, 1] = the roped 128-row chunk
                (h0 rope rows 0:64, h1 rows 64:128) -- written identically for
                both heads, the stationary zero rows select the right half."""
                row0 = b * T + qc * BLK
                tok = slice(qc * BLK, (qc + 1) * BLK)

                xt8c = work.tile([128, 8, 2, BLK], fp8, tag="xt8c", bufs=2)
                nc.sync.dma_start(
                    xt8c[:],
                    xt8_d[:, row0 : row0 + BLK].rearrange(
                        "(kp two p) t -> p kp two t", p=128, two=2
                    ),
                )

                qf8 = work.tile([128, 2, 2, BLK], fp8, tag="qf8", bufs=8)
                rot = work.tile([128, BLK], bf16, tag="rot", bufs=2)
                for mc in range(3):
                    pq = ps.tile([128, BLK], f32, tag="proj", bufs=4, name="pq")
                    for kp in range(8):
                        nc.tensor.matmul(
                            pq[:],
                            w1_sb[:, kp, mc, :, :],
                            xt8c[:, kp, :, :],
                            start=(kp == 0),
                            stop=(kp == 7),
                            perf_mode=DR,
                        )
                    if mc < 2:
                        nc.vector.tensor_copy(qf8[:, mc, 0, :], pq[:])
                    else:
                        for hh in range(2):
                            r0 = hh * 64
                            nc.vector.tensor_scalar_mul(
                                rot[r0 : r0 + 32, :], pq[r0 + 32 : r0 + 64, :], -1.0
                            )
                            nc.vector.tensor_copy(
                                rot[r0 + 32 : r0 + 64, :], pq[r0 : r0 + 32, :]
                            )
                        qre = work.tile([128, BLK], bf16, tag="qre", bufs=2)
                        nc.vector.tensor_mul(out=qre[:], in0=pq[:], in1=cs_sb[:, tok])
                        nc.vector.tensor_mul(out=rot[:], in0=rot[:], in1=sn_sb[:, tok])
                        nc.vector.tensor_add(out=qre[:], in0=qre[:], in1=rot[:])
                        nc.vector.tensor_copy(qf8[:, 0, 1, :], qre[:])
                        nc.vector.tensor_copy(qf8[:, 1, 1, :], qre[:])
                return qf8

            def stage_a2(b, qc, kv):
                """post-AG: rms norm, kv up-projection (per-arriving latent
                tile), k rope; k stored fp8 in score layout."""
                kn8, vnat = kv
                src = b * 4 + qc
                tok = slice(qc * BLK, (qc + 1) * BLK)

                ssq = ps.tile([1, BLK], f32, tag="xps", bufs=1, name="ssq")
                pkv = [
                    ps.tile([128, BLK], f32, tag="proj", bufs=4, name="pkvp")
                    for _ in range(4)
                ]
                for kc in range(4):
                    latk = work.tile([128, BLK], bf16, tag="latk", bufs=4)
                    nc.scalar.dma_start(
                        latk[:], ag_out[kc][src * 128 : (src + 1) * 128, :]
                    )
                    sqc = work.tile([128, BLK], bf16, tag="sqc", bufs=2)
                    nc.scalar.activation(sqc[:], latk[:], SQUARE)
                    nc.tensor.matmul(
                        ssq[:], ones_b[:], sqc[:], start=(kc == 0), stop=(kc == 3)
                    )
                    for mc in range(4):  # [h0 nope, h0 v, h1 nope, h1 v]
                        nc.tensor.matmul(
                            pkv[mc][:],
                            wb_sb[:, kc, mc * 128 : (mc + 1) * 128],
                            latk[:],
                            start=(kc == 0),
                            stop=(kc == 3),
                        )
                kraw = work.tile([ROPE, BLK], bf16, tag="kraw", bufs=2)
                nc.scalar.dma_start(kraw[:], ag_out[4][src * 64 : (src + 1) * 64, :])

                # rms scale: 1/sqrt(ssq/512+eps) = exp(-0.5*ln(.))
                lnrow = work.tile([1, BLK], f32, tag="lnrow", bufs=1)
                nc.scalar.activation(lnrow[:], ssq[:], LN, bias=eps[:], scale=1.0 / KVR)
                invrow = work.tile([1, BLK], bf16, tag="invrow", bufs=1)
                nc.scalar.activation(invrow[:], lnrow[:], EXP, scale=-0.5)
                invbc_ps = ps.tile([128, BLK], f32, tag="xps", bufs=1, name="invbc_ps")
                nc.tensor.matmul(invbc_ps[:], onesrow_b[:], invrow[:])
                invbc = work.tile([128, BLK], bf16, tag="invbc", bufs=2)
                nc.vector.tensor_copy(invbc[:], invbc_ps[:])

                # k rope from gathered raw rows -> both heads' fp8 tiles
                rot = work.tile([128, BLK], bf16, tag="rot", bufs=2)
                kr = work.tile([ROPE, BLK], bf16, tag="krw", bufs=2)
                nc.vector.tensor_scalar_mul(rot[0:32, :], kraw[32:64, :], -1.0)
                nc.vector.tensor_copy(rot[32:64, :], kraw[0:32, :])
                nc.vector.tensor_mul(out=kr[:], in0=kraw[:], in1=cs_sb[0:64, tok])
                nc.vector.tensor_mul(
                    out=rot[0:64, :], in0=rot[0:64, :], in1=sn_sb[0:64, tok]
                )
                nc.vector.tensor_add(out=kr[:], in0=kr[:], in1=rot[0:64, :])
                kt4 = slice(qc * 4, qc * 4 + 4)
                nc.vector.tensor_copy(kn8[0][0:64, kt4, 1, :], kr[:])
                nc.vector.tensor_copy(kn8[1][64:128, kt4, 1, :], kr[:])

                # normalize + store k_nope (fp8) and v (bf16, natural layout)
                for mc in range(4):
                    h = mc // 2
                    if mc % 2 == 0:
                        nc.vector.tensor_mul(
                            out=kn8[h][:, kt4, 0, :], in0=pkv[mc][:], in1=invbc[:]
                        )
                    else:
                        vuT = work.tile([128, BLK], bf16, tag="vuT", bufs=2)
                        nc.vector.tensor_mul(out=vuT[:], in0=pkv[mc][:], in1=invbc[:])
                        nc.sync.dma_start_transpose(
                            vnat[h][:, qc * 4 : qc * 4 + 4, :], vuT[:]
                        )

            def stage_bh(b, qc, h, qf8, kv):
                """causal attention for one q-chunk, one head.

                One fp8 DoubleRow matmul per score tile (k_nope + padded
                k_rope contract together). exp/mask/acc/yacc trim to the
                valid diagonal column ranges. Lookahead-2; denominator on
                Vector in f32r."""
                kn8, vnat = kv
                n_kt = 4 * (qc + 1)
                yacc = ps.tile([VD, BLK], f32, tag="yacc", bufs=1)
                acc = work.tile([128, BLK], f32r, tag="acc", bufs=2)
                pend = []
                for kt in range(n_kt):
                    j = kt - 4 * qc
                    c0 = max(j, 0) * 128  # first valid q column
                    qs = slice(c0, BLK)
                    st = ps.tile([128, BLK], f32, tag="st", bufs=2)
                    nc.tensor.matmul(
                        st[:], kn8[h][:, kt, :, :], qf8[:, h, :, :],
                        start=True, stop=True, perf_mode=DR,
                    )
                    if len(pend) == 2:
                        pe_est, pk, pqs = pend.pop(0)
                        nc.tensor.matmul(
                            yacc[:, pqs], vnat[h][:, pk, :], pe_est[:, pqs],
                            start=(pk == 0), stop=False,
                        )
                    est = work.tile([128, BLK], bf16, tag="est", bufs=4)
                    nc.scalar.activation(est[:, qs], st[:, qs], EXP, scale=1.0 / S)
                    if j >= 0:
                        nc.vector.tensor_mul(
                            out=est[:, qs], in0=est[:, qs],
                            in1=msk_sb[:, j * BLK + c0 : (j + 1) * BLK],
                        )
                    if kt == 0:
                        nc.vector.tensor_copy(acc[:], est[:])
                    else:
                        nc.vector.tensor_add(
                            out=acc[:, qs], in0=acc[:, qs], in1=est[:, qs]
                        )
                    pend.append((est, kt, qs))
                for pe_est, pk, pqs in pend:
                    nc.tensor.matmul(
                        yacc[:, pqs], vnat[h][:, pk, :], pe_est[:, pqs],
                        start=(pk == 0), stop=(pk == n_kt - 1),
                    )

                sums = ps.tile([1, BLK], f32, tag="xps", bufs=1, name="sums")
                nc.tensor.matmul(sums[:], ones_r[:], acc[:])
                lnr = work.tile([1, BLK], f32, tag="lnrow", bufs=1)
                nc.scalar.activation(lnr[:], sums[:], LN)
                sinvrow = work.tile([1, BLK], bf16, tag="invrow", bufs=1)
                nc.scalar.activation(sinvrow[:], lnr[:], EXP, scale=-1.0)
                sbc_ps = ps.tile([128, BLK], f32, tag="xps", bufs=1, name="sbc_ps")
                nc.tensor.matmul(sbc_ps[:], onesrow_b[:], sinvrow[:])
                sinv = work.tile([128, BLK], bf16, tag="sinv", bufs=1)
                nc.vector.tensor_copy(sinv[:], sbc_ps[:])
                ysb = work.tile([VD, BLK], bf16, tag="ysb", bufs=1)
                nc.vector.tensor_mul(out=ysb[:], in0=yacc[:], in1=sinv[:])
                for jj in range(2):
                    nc.sync.dma_start(
                        y_in[b][qc * 2 + jj, h * VD : (h + 1) * VD, :],
                        ysb[:, jj * 256 : (jj + 1) * 256],
                    )

            def emit_a2a(b):
                nc.gpsimd.collective_compute(
                    "AllToAll",
                    mybir.AluOpType.bypass,
                    replica_groups=RG,
                    ins=[y_in[b].opt()],
                    outs=[y_out[b].opt()],
                )

            a2a_sb = {}

            def wo_load(b):
                a2a = wop.tile([128, 16, 256], bf16, tag="a2a", bufs=1, name=f"a2a{b}")
                nc.gpsimd.dma_start(
                    a2a[:], y_out[b][:].rearrange("c (h p) t -> p (c h) t", p=128)
                )
                a2a_sb[b] = a2a

            def wo_chains(b, chains):
                """wo projection chains for this batch's gathered token slice."""
                a2a = a2a_sb[b]
                for n, tt in chains:
                    pout = ps.tile([128, 512], f32, tag="proj", bufs=4, name="pout")
                    for kc in range(16):
                        nc.tensor.matmul(
                            pout[:],
                            a2a[:, kc, tt * 128 : (tt + 1) * 128],
                            wo_box["wo"][:, kc, n * 512 : (n + 1) * 512],
                            start=(kc == 0),
                            stop=(kc == 15),
                        )
                    osb = wop.tile([128, 512], bf16, tag="osb", bufs=2)
                    nc.vector.tensor_copy(osb[:], pout[:])
                    nc.sync.dma_start(
                        out_d[
                            b, tt * 128 : (tt + 1) * 128, n * 512 : (n + 1) * 512
                        ],
                        osb[:],
                    )

            ALLC = [(n, tt) for n in range(4) for tt in range(2)]

            # ---- schedule (Tile reorders by dependencies; emission order
            # mostly matters for same-engine queue order) ----
            kv0 = alloc_kv(0)
            kv1 = alloc_kv(1)
            kvc_block()
            load_msk()
            q00 = stage_a1(0, 0)
            q01 = stage_a1(0, 1)
            q02 = stage_a1(0, 2)
            q03 = stage_a1(0, 3)
            stage_a2(0, 0, kv0)
            q10 = stage_a1(1, 0)
            stage_a2(0, 1, kv0)
            q11 = stage_a1(1, 1)
            stage_bh(0, 0, 0, q00, kv0)
            q12 = stage_a1(1, 2)
            stage_bh(0, 0, 1, q00, kv0)
            load_wo()
            q13 = stage_a1(1, 3)
            stage_a2(0, 2, kv0)
            stage_bh(0, 1, 0, q01, kv0)
            stage_bh(0, 1, 1, q01, kv0)
            stage_a2(0, 3, kv0)
            stage_bh(0, 2, 0, q02, kv0)
            stage_bh(0, 2, 1, q02, kv0)
            stage_bh(0, 3, 0, q03, kv0)
            stage_a2(1, 0, kv1)
            stage_bh(0, 3, 1, q03, kv0)
            emit_a2a(0)
            stage_a2(1, 1, kv1)
            stage_bh(1, 0, 0, q10, kv1)
            stage_bh(1, 0, 1, q10, kv1)
            stage_a2(1, 2, kv1)
            wo_load(0)
            wo_chains(0, ALLC[0:2])
            stage_bh(1, 1, 0, q11, kv1)
            stage_bh(1, 1, 1, q11, kv1)
            stage_a2(1, 3, kv1)
            wo_chains(0, ALLC[2:4])
            stage_bh(1, 2, 0, q12, kv1)
            stage_bh(1, 2, 1, q12, kv1)
            stage_bh(1, 3, 0, q13, kv1)
            stage_bh(1, 3, 1, q13, kv1)
            emit_a2a(1)
            wo_chains(0, ALLC[4:8])
            wo_load(1)
            wo_chains(1, ALLC)

    nc.compile()
    return nc


def host_prep(x, wq, wkv_a, wkv_b, wo, kv_norm_w):
    bf = ml_dtypes.bfloat16
    f8 = ml_dtypes.float8_e4m3fn
    scale = np.float32(QKD ** -0.5)
    inv = (1.0 / (10000.0 ** (np.arange(0, ROPE, 2, dtype=np.float32) / ROPE))).astype(
        np.float32
    )
    f = np.outer(np.arange(T, dtype=np.float32), inv)
    cos32 = np.cos(f).T.astype(np.float32)
    sin32 = np.sin(f).T.astype(np.float32)
    cos128 = np.ascontiguousarray(np.concatenate([cos32] * 4, 0)).astype(bf)
    sin128 = np.ascontiguousarray(np.concatenate([sin32] * 4, 0)).astype(bf)
    wkv_bw = (wkv_b * kv_norm_w[:, None]).astype(np.float32)
    xtf = np.ascontiguousarray(x.reshape(B * T, D).T)
    xt8 = xtf.astype(f8)
    wa = np.ascontiguousarray(wkv_a).astype(bf)
    wo_c = np.ascontiguousarray(wo).astype(bf)
    wq_r = wq.reshape(D, H, QKD)

    kk = np.arange(128)[:, None]
    qq = np.arange(BLK)[None, :]
    msk = np.concatenate(
        [(qq >= kk + j * 128).astype(np.float32) for j in range(4)], axis=1
    ).astype(bf)

    in_maps = []
    for c in range(NCORES):
        h0 = HPC * c
        w1f = np.concatenate(
            [
                wq_r[:, h0, :NOPE],
                wq_r[:, h0 + 1, :NOPE],
                wq_r[:, h0, NOPE:],
                wq_r[:, h0 + 1, NOPE:],
            ],
            axis=1,
        ) * (scale * S)
        # permute to rows (kp, p), cols (mt, two, mw): each DoubleRow
        # stationary slice [p, two, mw] must be contiguous in SBUF
        w1p = w1f.reshape(8, 2, 128, 3, 128).transpose(0, 2, 3, 1, 4)
        w1 = np.ascontiguousarray(w1p.reshape(1024, 768)).astype(f8)
        xslice = np.ascontiguousarray(xtf[:, c * BLK : (c + 1) * BLK]).astype(bf)
        wb = np.ascontiguousarray(
            wkv_bw[:, h0 * (NOPE + VD) : (h0 + 2) * (NOPE + VD)]
        ).astype(bf)
        in_maps.append(
            {
                "xt8": xt8,
                "xs": xslice,
                "w1": w1,
                "wa": wa,
                "wb": wb,
                "wo": wo_c,
                "cos": cos128,
                "sin": sin128,
                "msk": msk,
            }
        )
    return in_maps


_NC = None


def kernel(x, wq, wkv_a, wkv_b, wo, kv_norm_w, _trace=False):
    global _NC
    if _NC is None:
        _NC = build_program()
    in_maps = host_prep(
        np.asarray(x, np.float32),
        np.asarray(wq, np.float32),
        np.asarray(wkv_a, np.float32),
        np.asarray(wkv_b, np.float32),
        np.asarray(wo, np.float32),
        np.asarray(kv_norm_w, np.float32),
    )
    res = run_bass_kernel_spmd(_NC, in_maps, list(range(NCORES)), trace=_trace)
    out = np.empty((B, T, D), np.float32)
    cw = T // NCORES
    for c in range(NCORES):
        oc = res.results[c]["out"].astype(np.float32)  # (B, 256, D)
        for b in range(B):
            out[b, c * cw : (c + 1) * cw, :] = oc[b]
    kernel.last_results = res
    return out


# revision 36
# speedup vs baseline: 1.0166x; 1.0166x over previous
"""Multi-Head Latent Attention (MLA) forward on 8 Trainium2 NeuronCores.

Sharding: tensor-parallel over heads (16 heads -> 2 per core), with the
latent-kv encoding sharded by TOKENS:
  - each core computes the full 576-dim kvc for its own 512-token slice
    from a 2MB bf16 slice of x.T; the result is distributed through FIVE
    pipelined AllGathers (one per 128-column tile), and the kv
    up-projection consumes each latent tile as it arrives,
  - q projections run as fp8e4 DoubleRow matmuls (K=256 per pass, 2x PE
    throughput) on an fp8 copy of x.T; the fp8 weight scale S is undone
    for free by the softmax exp's scale argument (exp(st/S)),
  - the latent path stays bf16 through the up-projection; k (nope+rope)
    and q are then quantized to fp8 so each score tile is a single
    DoubleRow matmul: k-tile 0 = k_nope, k-tile 1 = k_rope zero-padded,
    with per-head zero masks selecting that head's rope rows,
  - causal attention in transposed-score layout; exp/mask/denominator/
    attention-v work only on the valid lower-triangular column ranges of
    diagonal tiles; the denominator accumulates on the Vector engine in
    f32r so neither the PE nor the slow GpSimd gates the chain,
  - per-batch AllToAll exchanges head outputs; wo loads mid-flight and the
    last AllToAll is drained with a single rearranged DMA.
Output slices are disjoint; the host just concatenates them.
"""
import sys

if "/opt/trn_rl_repo" not in sys.path:
    sys.path.insert(0, "/opt/trn_rl_repo")

import numpy as np
import ml_dtypes
import concourse.bacc as bacc
import concourse.mybir as mybir
from concourse import tile
from concourse.bass_utils import run_bass_kernel_spmd

H, NOPE, ROPE, VD, KVR, QKD = 16, 128, 64, 128, 512, 192
B, T, D = 2, 2048, 2048
NCORES, HPC, BLK = 8, 2, 512
KVC = KVR + ROPE  # 576 latent+rope columns
S = 128.0  # fp8 weight scale, undone inside the softmax exp
f32 = mybir.dt.float32
f32r = mybir.dt.float32r
bf16 = mybir.dt.bfloat16
fp8 = mybir.dt.float8e4
DR = mybir.MatmulPerfMode.DoubleRow
EXP = mybir.ActivationFunctionType.Exp
LN = mybir.ActivationFunctionType.Ln
SQUARE = mybir.ActivationFunctionType.Square


def _patch_act_tables():
    """Make the act-table-load pass serve Exp/Ln/Square from the one set that
    contains them all (natural_log_exp_and_others), so interleaved activations
    don't thrash table loads. Indices into act_info.json must be preserved, so
    the shadowing single-function sets are emptied in place, not removed."""
    import concourse.bacc as _bacc

    orig = _bacc.get_activation_tables
    if getattr(_bacc, "_mla_act_patch", False):
        return
    _bacc._mla_act_patch = True

    def patched(arch):
        d = dict(orig(arch))
        if "natural_log_exp_and_others" in d:
            for name in ("exp_and_others", "natural_log", "exp_and_friends"):
                if name in d:
                    d[name] = set()
        return d

    _bacc.get_activation_tables = patched


def build_program():
    _patch_act_tables()
    nc = bacc.Bacc("TRN2", target_bir_lowering=False, debug=False, num_devices=NCORES)
    xt8_d = nc.dram_tensor("xt8", [D, B * T], fp8, kind="ExternalInput")
    xs_d = nc.dram_tensor("xs", [D, BLK], bf16, kind="ExternalInput")
    xs01_d = nc.dram_tensor("xs01", [D, 2 * BLK], bf16, kind="ExternalInput")
    # w1 pre-permuted on host: rows (kp, p), cols (mt, two, mw) so each
    # DoubleRow stationary slice [128, 2, 128] is contiguous (ISA requirement)
    w1_d = nc.dram_tensor("w1", [D // 2, 768], fp8, kind="ExternalInput")
    wa_d = nc.dram_tensor("wa", [D, KVC], bf16, kind="ExternalInput")
    wb_d = nc.dram_tensor("wb", [KVR, HPC * (NOPE + VD)], bf16, kind="ExternalInput")
    wo_d = nc.dram_tensor("wo", [H * VD, D], bf16, kind="ExternalInput")
    cos_d = nc.dram_tensor("cos", [128, T], bf16, kind="ExternalInput")
    sin_d = nc.dram_tensor("sin", [128, T], bf16, kind="ExternalInput")
    msk_d = nc.dram_tensor("msk", [128, 4 * BLK], bf16, kind="ExternalInput")
    out_d = nc.dram_tensor("out", [B, T // NCORES, D], bf16, kind="ExternalOutput")

    RG = [list(range(NCORES))]

    with tile.TileContext(nc) as tc:
        with (
            tc.tile_pool(name="dram", bufs=1, space="DRAM") as dram,
            tc.tile_pool(name="const", bufs=1) as const,
            tc.tile_pool(name="wpool", bufs=1) as wpool,
            tc.tile_pool(name="kvpool", bufs=1) as kvpool,
            tc.tile_pool(name="work", bufs=1) as work,
            tc.tile_pool(name="wop", bufs=1) as wop,
            tc.tile_pool(name="ps", bufs=1, space="PSUM") as ps,
        ):
            y_in = [
                dram.tile([NCORES, HPC * VD, 256], bf16, name=f"y_in{b}")
                for b in range(B)
            ]
            y_out = [
                dram.tile([NCORES, HPC * VD, 256], bf16, name=f"y_out{b}")
                for b in range(B)
            ]
            MTS = [128, 128, 128, 128, 64]
            ag_in = dram.tile([KVC, BLK], bf16, name="ag_in")
            ag_out = dram.tile(
                [NCORES * KVC, BLK], bf16, name="ag_out", addr_space="Shared"
            )

            agd_in = dram.tile([1, 64], bf16, name="agd_in")
            agd_out = dram.tile([8, 64], bf16, name="agd_out", addr_space="Shared")

            ones_f = const.tile([128, 1], f32, tag="ones_f")
            nc.gpsimd.memset(ones_f[:], 1.0)
            ones_r = const.tile([128, 1], f32r, tag="ones_r")
            nc.vector.tensor_copy(ones_r[:], ones_f[:])
            ones_b = const.tile([128, 1], bf16, tag="ones_b")
            nc.vector.tensor_copy(ones_b[:], ones_f[:])
            onesrow_f = const.tile([1, 128], f32, tag="onesrow_f")
            nc.gpsimd.memset(onesrow_f[:], 1.0)
            onesrow_b = const.tile([1, 128], bf16, tag="onesrow_b")
            nc.vector.tensor_copy(onesrow_b[:], onesrow_f[:])
            eps = const.tile([1, 1], f32, tag="eps")
            nc.gpsimd.memset(eps[:], 1e-6)

            dummy_sb = const.tile([1, 64], bf16, tag="dummy")
            nc.gpsimd.memset(dummy_sb[:], 0.0)
            nc.sync.dma_start(agd_in[:], dummy_sb[:])
            nc.gpsimd.collective_compute(
                "AllGather",
                mybir.AluOpType.bypass,
                replica_groups=RG,
                ins=[agd_in.opt()],
                outs=[agd_out.opt()],
            )

            # xs shares its SBUF slot with the later a2a staging tiles, and wa
            # with the big wo tile: both are dead before their partner loads.
            # Loads split into kc quarters so the kvc matmuls start ~4us in.
            xs_sb = wop.tile([128, 16, BLK], bf16, tag="a2a", bufs=1, name="xs")
            wa_sb = wpool.tile([128, 16, KVC], bf16, tag="wavo", bufs=1, name="wa")
            for g in range(4):
                kcs = slice(g * 4, g * 4 + 4)
                rows = slice(g * 512, (g + 1) * 512)
                nc.sync.dma_start(
                    xs_sb[:, kcs, :],
                    xs_d[rows, :].rearrange("(kc p) t -> p kc t", p=128),
                )
                nc.sync.dma_start(
                    wa_sb[:, kcs, :],
                    wa_d[rows, :].rearrange("(kc p) m -> p kc m", p=128),
                )
            w1_sb = wpool.tile([128, 8, 3, 2, 128], fp8, tag="w1")
            nc.sync.dma_start(
                w1_sb[:],
                w1_d[:].rearrange(
                    "(kp p) (mt two mw) -> p kp mt two mw", p=128, mt=3, two=2
                ),
            )
            wb_sb = wpool.tile([128, 4, 512], bf16, tag="wb")
            nc.sync.dma_start(wb_sb[:], wb_d[:].rearrange("(kc p) m -> p kc m", p=128))
            cs_sb = wpool.tile([128, T], bf16, tag="cs")
            nc.sync.dma_start(cs_sb[:], cos_d[:])
            sn_sb = wpool.tile([128, T], bf16, tag="sn")
            nc.sync.dma_start(sn_sb[:], sin_d[:])
            msk_sb = wpool.tile([128, 4 * BLK], bf16, tag="msk")

            def load_msk():
                nc.sync.dma_start(msk_sb[:], msk_d[:])

            wo_box = {}

            def load_wo():
                wo_box["wo"] = wpool.tile(
                    [128, 16, D], bf16, tag="wavo", bufs=1, name="wo"
                )
                nc.sync.dma_start(
                    wo_box["wo"][:], wo_d[:].rearrange("(kc p) m -> p kc m", p=128)
                )

            def kvc_block():
                """Full 576-col kvc for this core's 512-token slice + one AG."""
                for mt in range(5):
                    m0 = mt * 128
                    mw = MTS[mt]
                    pc = ps.tile([128, BLK], f32, tag="proj", bufs=4, name="pc")
                    for kc in range(16):
                        nc.tensor.matmul(
                            pc[:mw, :],
                            wa_sb[:, kc, m0 : m0 + mw],
                            xs_sb[:, kc, :],
                            start=(kc == 0),
                            stop=(kc == 15),
                        )
                    kvcm = work.tile([128, BLK], bf16, tag="kvcm", bufs=2)
                    nc.vector.tensor_copy(kvcm[:mw, :], pc[:mw, :])
                    nc.sync.dma_start(ag_in[m0 : m0 + mw, :], kvcm[:mw, :])
                nc.gpsimd.collective_compute(
                    "AllGather",
                    mybir.AluOpType.bypass,
                    replica_groups=RG,
                    ins=[ag_in.opt()],
                    outs=[ag_out.opt()],
                )

            def rep_kvc(r):
                """Recompute chunk (0, r)'s 576-col kvc locally (every core) so
                batch-0 chunks 0/1 attention runs before the AG lands."""
                pk = [
                    ps.tile([128, BLK], f32, tag="proj", bufs=4, name="rkp")
                    for _ in range(4)
                ]
                pr = ps.tile([128, BLK], f32, tag="xps", bufs=1, name="rkr")
                for kc in range(16):
                    xr = work.tile([128, BLK], bf16, tag="xrk", bufs=4)
                    nc.sync.dma_start(
                        xr[:],
                        xs01_d[kc * 128 : (kc + 1) * 128, r * BLK : (r + 1) * BLK],
                    )
                    for mt in range(4):
                        nc.tensor.matmul(
                            pk[mt][:],
                            wa_sb[:, kc, mt * 128 : (mt + 1) * 128],
                            xr[:],
                            start=(kc == 0),
                            stop=(kc == 15),
                        )
                    nc.tensor.matmul(
                        pr[:64, :],
                        wa_sb[:, kc, 512:],
                        xr[:],
                        start=(kc == 0),
                        stop=(kc == 15),
                    )
                latks = []
                for mt in range(4):
                    latk = work.tile([128, BLK], bf16, tag="latk", bufs=4)
                    nc.vector.tensor_copy(latk[:], pk[mt][:])
                    latks.append(latk)
                kraw = work.tile([ROPE, BLK], bf16, tag="kraw", bufs=2)
                nc.vector.tensor_copy(kraw[:], pr[:64, :])
                return latks, kraw

            # per-batch kv staging in fp8 score layout: kn8[h] holds k-tile 0
            # (k_nope) and k-tile 1 (k_rope in that head's partition half,
            # zeros elsewhere); vnat stays bf16 for the attention-v matmul
            def alloc_kv(b):
                kn8 = [
                    kvpool.tile(
                        [128, 16, 2, 128], fp8, tag="kn8", bufs=2, name=f"kn8{b}_{h}"
                    )
                    for h in range(2)
                ]
                vnat = [
                    kvpool.tile(
                        [128, 16, VD], bf16, tag="vnat", bufs=2, name=f"vnat{b}_{h}"
                    )
                    for h in range(2)
                ]
                nc.gpsimd.memset(kn8[0][64:128, :, 1, :], 0.0)
                nc.gpsimd.memset(kn8[1][0:64, :, 1, :], 0.0)
                return kn8, vnat

            def stage_a1(b, qc):
                """fp8 x chunk load + DoubleRow q projection + q rope.

                qf8[h, 0] = S*q_nope(h); qf8[*, 1] = the roped 128-row chunk
                (h0 rope rows 0:64, h1 rows 64:128) -- written identically for
                both heads, the stationary zero rows select the right half."""
                row0 = b * T + qc * BLK
                tok = slice(qc * BLK, (qc + 1) * BLK)

                xt8c = work.tile([128, 8, 2, BLK], fp8, tag="xt8c", bufs=2)
                nc.sync.dma_start(
                    xt8c[:],
                    xt8_d[:, row0 : row0 + BLK].rearrange(
                        "(kp two p) t -> p kp two t", p=128, two=2
                    ),
                )

                qf8 = work.tile([128, 2, 2, BLK], fp8, tag="qf8", bufs=8)
                rot = work.tile([128, BLK], bf16, tag="rot", bufs=2)
                for mc in range(3):
                    pq = ps.tile([128, BLK], f32, tag="proj", bufs=4, name="pq")
                    for kp in range(8):
                        nc.tensor.matmul(
                            pq[:],
                            w1_sb[:, kp, mc, :, :],
                            xt8c[:, kp, :, :],
                            start=(kp == 0),
                            stop=(kp == 7),
                            perf_mode=DR,
                        )
                    if mc < 2:
                        nc.vector.tensor_copy(qf8[:, mc, 0, :], pq[:])
                    else:
                        for hh in range(2):
                            r0 = hh * 64
                            nc.vector.tensor_scalar_mul(
                                rot[r0 : r0 + 32, :], pq[r0 + 32 : r0 + 64, :], -1.0
                            )
                            nc.vector.tensor_copy(
                                rot[r0 + 32 : r0 + 64, :], pq[r0 : r0 + 32, :]
                            )
                        qre = work.tile([128, BLK], bf16, tag="qre", bufs=2)
                        nc.vector.tensor_mul(out=qre[:], in0=pq[:], in1=cs_sb[:, tok])
                        nc.vector.tensor_mul(out=rot[:], in0=rot[:], in1=sn_sb[:, tok])
                        nc.vector.tensor_add(out=qre[:], in0=qre[:], in1=rot[:])
                        nc.vector.tensor_copy(qf8[:, 0, 1, :], qre[:])
                        nc.scalar.activation(
                            qf8[:, 1, 1, :], qre[:],
                            mybir.ActivationFunctionType.Copy,
                        )
                return qf8

            def stage_a2(b, qc, kv, local=None):
                """post-AG (or from local replication): rms norm, kv
                up-projection, k rope; k stored fp8 in score layout."""
                kn8, vnat = kv
                src = b * 4 + qc
                tok = slice(qc * BLK, (qc + 1) * BLK)

                ssq = ps.tile([1, BLK], f32, tag="xps", bufs=1, name="ssq")
                pkv = [
                    ps.tile([128, BLK], f32, tag="proj", bufs=4, name="pkvp")
                    for _ in range(4)
                ]
                for kc in range(4):
                    if local is None:
                        latk = work.tile([128, BLK], bf16, tag="latk", bufs=4)
                        nc.scalar.dma_start(
                            latk[:],
                            ag_out[src * KVC + kc * 128 : src * KVC + (kc + 1) * 128, :],
                        )
                    else:
                        latk = local[0][kc]
                    sqc = work.tile([128, BLK], bf16, tag="sqc", bufs=2)
                    nc.scalar.activation(sqc[:], latk[:], SQUARE)
                    nc.tensor.matmul(
                        ssq[:], ones_b[:], sqc[:], start=(kc == 0), stop=(kc == 3)
                    )
                    for mc in range(4):  # [h0 nope, h0 v, h1 nope, h1 v]
                        nc.tensor.matmul(
                            pkv[mc][:],
                            wb_sb[:, kc, mc * 128 : (mc + 1) * 128],
                            latk[:],
                            start=(kc == 0),
                            stop=(kc == 3),
                        )
                if local is None:
                    kraw = work.tile([ROPE, BLK], bf16, tag="kraw", bufs=2)
                    nc.scalar.dma_start(
                        kraw[:], ag_out[src * KVC + KVR : (src + 1) * KVC, :]
                    )
                else:
                    kraw = local[1]

                # rms scale: 1/sqrt(ssq/512+eps) = exp(-0.5*ln(.))
                lnrow = work.tile([1, BLK], f32, tag="lnrow", bufs=1)
                nc.scalar.activation(lnrow[:], ssq[:], LN, bias=eps[:], scale=1.0 / KVR)
                invrow = work.tile([1, BLK], bf16, tag="invrow", bufs=1)
                nc.scalar.activation(invrow[:], lnrow[:], EXP, scale=-0.5)
                invbc_ps = ps.tile([128, BLK], f32, tag="xps", bufs=1, name="invbc_ps")
                nc.tensor.matmul(invbc_ps[:], onesrow_b[:], invrow[:])
                invbc = work.tile([128, BLK], bf16, tag="invbc", bufs=2)
                nc.vector.tensor_copy(invbc[:], invbc_ps[:])

                # k rope from gathered raw rows -> both heads' fp8 tiles
                rot = work.tile([128, BLK], bf16, tag="rot", bufs=2)
                kr = work.tile([ROPE, BLK], bf16, tag="krw", bufs=2)
                nc.vector.tensor_scalar_mul(rot[0:32, :], kraw[32:64, :], -1.0)
                nc.vector.tensor_copy(rot[32:64, :], kraw[0:32, :])
                nc.vector.tensor_mul(out=kr[:], in0=kraw[:], in1=cs_sb[0:64, tok])
                nc.vector.tensor_mul(
                    out=rot[0:64, :], in0=rot[0:64, :], in1=sn_sb[0:64, tok]
                )
                nc.vector.tensor_add(out=kr[:], in0=kr[:], in1=rot[0:64, :])
                kt4 = slice(qc * 4, qc * 4 + 4)
                nc.vector.tensor_copy(kn8[0][0:64, kt4, 1, :], kr[:])
                nc.scalar.activation(
                    kn8[1][64:128, kt4, 1, :], kr[:],
                    mybir.ActivationFunctionType.Copy,
                )

                # normalize + store k_nope (fp8) and v (bf16, natural layout)
                for mc in range(4):
                    h = mc // 2
                    if mc % 2 == 0:
                        nc.vector.tensor_mul(
                            out=kn8[h][:, kt4, 0, :], in0=pkv[mc][:], in1=invbc[:]
                        )
                    else:
                        vuT = work.tile([128, BLK], bf16, tag="vuT", bufs=2)
                        nc.vector.tensor_mul(out=vuT[:], in0=pkv[mc][:], in1=invbc[:])
                        nc.sync.dma_start_transpose(
                            vnat[h][:, qc * 4 : qc * 4 + 4, :], vuT[:]
                        )

            def stage_bh(b, qc, h, qf8, kv):
                """causal attention for one q-chunk, one head.

                One fp8 DoubleRow matmul per score tile (k_nope + padded
                k_rope contract together). exp/mask/acc/yacc trim to the
                valid diagonal column ranges. Lookahead-2; denominator on
                Vector in f32r."""
                kn8, vnat = kv
                n_kt = 4 * (qc + 1)
                yacc = ps.tile([VD, BLK], f32, tag="yacc", bufs=1)
                acc = work.tile([128, BLK], f32r, tag="acc", bufs=2)
                pend = []
                for kt in range(n_kt):
                    j = kt - 4 * qc
                    c0 = max(j, 0) * 128  # first valid q column
                    qs = slice(c0, BLK)
                    st = ps.tile([128, BLK], f32, tag="st", bufs=2)
                    nc.tensor.matmul(
                        st[:], kn8[h][:, kt, :, :], qf8[:, h, :, :],
                        start=True, stop=True, perf_mode=DR,
                    )
                    if len(pend) == 2:
                        pe_est, pk, pqs = pend.pop(0)
                        nc.tensor.matmul(
                            yacc[:, pqs], vnat[h][:, pk, :], pe_est[:, pqs],
                            start=(pk == 0), stop=False,
                        )
                    est = work.tile([128, BLK], bf16, tag="est", bufs=4)
                    nc.scalar.activation(est[:, qs], st[:, qs], EXP, scale=1.0 / S)
                    if j >= 0:
                        nc.vector.tensor_mul(
                            out=est[:, qs], in0=est[:, qs],
                            in1=msk_sb[:, j * BLK + c0 : (j + 1) * BLK],
                        )
                    if kt == 0:
                        nc.vector.tensor_copy(acc[:], est[:])
                    else:
                        nc.vector.tensor_add(
                            out=acc[:, qs], in0=acc[:, qs], in1=est[:, qs]
                        )
                    pend.append((est, kt, qs))
                for pe_est, pk, pqs in pend:
                    nc.tensor.matmul(
                        yacc[:, pqs], vnat[h][:, pk, :], pe_est[:, pqs],
                        start=(pk == 0), stop=(pk == n_kt - 1),
                    )

                sums = ps.tile([1, BLK], f32, tag="xps", bufs=1, name="sums")
                nc.tensor.matmul(sums[:], ones_r[:], acc[:])
                lnr = work.tile([1, BLK], f32, tag="lnrow", bufs=1)
                nc.scalar.activation(lnr[:], sums[:], LN)
                sinvrow = work.tile([1, BLK], bf16, tag="invrow", bufs=1)
                nc.scalar.activation(sinvrow[:], lnr[:], EXP, scale=-1.0)
                sbc_ps = ps.tile([128, BLK], f32, tag="xps", bufs=1, name="sbc_ps")
                nc.tensor.matmul(sbc_ps[:], onesrow_b[:], sinvrow[:])
                sinv = work.tile([128, BLK], bf16, tag="sinv", bufs=1)
                nc.vector.tensor_copy(sinv[:], sbc_ps[:])
                ysb = work.tile([VD, BLK], bf16, tag="ysb", bufs=1)
                nc.vector.tensor_mul(out=ysb[:], in0=yacc[:], in1=sinv[:])
                for jj in range(2):
                    nc.sync.dma_start(
                        y_in[b][qc * 2 + jj, h * VD : (h + 1) * VD, :],
                        ysb[:, jj * 256 : (jj + 1) * 256],
                    )

            def emit_a2a(b):
                nc.gpsimd.collective_compute(
                    "AllToAll",
                    mybir.AluOpType.bypass,
                    replica_groups=RG,
                    ins=[y_in[b].opt()],
                    outs=[y_out[b].opt()],
                )

            a2a_sb = {}

            def wo_load(b):
                a2a = wop.tile([128, 16, 256], bf16, tag="a2a", bufs=1, name=f"a2a{b}")
                nc.gpsimd.dma_start(
                    a2a[:], y_out[b][:].rearrange("c (h p) t -> p (c h) t", p=128)
                )
                a2a_sb[b] = a2a

            def wo_chains(b, chains):
                """wo projection chains for this batch's gathered token slice."""
                a2a = a2a_sb[b]
                for n, tt in chains:
                    pout = ps.tile([128, 512], f32, tag="proj", bufs=4, name="pout")
                    for kc in range(16):
                        nc.tensor.matmul(
                            pout[:],
                            a2a[:, kc, tt * 128 : (tt + 1) * 128],
                            wo_box["wo"][:, kc, n * 512 : (n + 1) * 512],
                            start=(kc == 0),
                            stop=(kc == 15),
                        )
                    osb = wop.tile([128, 512], bf16, tag="osb", bufs=2)
                    nc.vector.tensor_copy(osb[:], pout[:])
                    nc.sync.dma_start(
                        out_d[
                            b, tt * 128 : (tt + 1) * 128, n * 512 : (n + 1) * 512
                        ],
                        osb[:],
                    )

            ALLC = [(n, tt) for n in range(4) for tt in range(2)]

            # ---- schedule (Tile reorders by dependencies; emission order
            # mostly matters for same-engine queue order) ----
            kv0 = alloc_kv(0)
            kv1 = alloc_kv(1)
            kvc_block()
            load_msk()
            q00 = stage_a1(0, 0)
            q01 = stage_a1(0, 1)
            loc0 = rep_kvc(0)
            stage_a2(0, 0, kv0, local=loc0)
            loc1 = rep_kvc(1)
            stage_a2(0, 1, kv0, local=loc1)
            stage_bh(0, 0, 0, q00, kv0)
            q02 = stage_a1(0, 2)
            stage_bh(0, 0, 1, q00, kv0)
            q03 = stage_a1(0, 3)
            stage_bh(0, 1, 0, q01, kv0)
            q10 = stage_a1(1, 0)
            stage_bh(0, 1, 1, q01, kv0)
            q11 = stage_a1(1, 1)
            q12 = stage_a1(1, 2)
            load_wo()
            q13 = stage_a1(1, 3)
            stage_a2(0, 2, kv0)
            stage_bh(0, 2, 0, q02, kv0)
            stage_bh(0, 2, 1, q02, kv0)
            stage_a2(0, 3, kv0)
            stage_bh(0, 3, 0, q03, kv0)
            stage_a2(1, 0, kv1)
            stage_bh(0, 3, 1, q03, kv0)
            emit_a2a(0)
            stage_a2(1, 1, kv1)
            stage_bh(1, 0, 0, q10, kv1)
            stage_bh(1, 0, 1, q10, kv1)
            stage_a2(1, 2, kv1)
            wo_load(0)
            wo_chains(0, ALLC[0:2])
            stage_bh(1, 1, 0, q11, kv1)
            stage_bh(1, 1, 1, q11, kv1)
            stage_a2(1, 3, kv1)
            wo_chains(0, ALLC[2:4])
            stage_bh(1, 2, 0, q12, kv1)
            stage_bh(1, 2, 1, q12, kv1)
            stage_bh(1, 3, 0, q13, kv1)
            stage_bh(1, 3, 1, q13, kv1)
            emit_a2a(1)
            wo_chains(0, ALLC[4:8])
            wo_load(1)
            wo_chains(1, ALLC)

    nc.compile()
    return nc


def host_prep(x, wq, wkv_a, wkv_b, wo, kv_norm_w):
    bf = ml_dtypes.bfloat16
    f8 = ml_dtypes.float8_e4m3fn
    scale = np.float32(QKD ** -0.5)
    inv = (1.0 / (10000.0 ** (np.arange(0, ROPE, 2, dtype=np.float32) / ROPE))).astype(
        np.float32
    )
    f = np.outer(np.arange(T, dtype=np.float32), inv)
    cos32 = np.cos(f).T.astype(np.float32)
    sin32 = np.sin(f).T.astype(np.float32)
    cos128 = np.ascontiguousarray(np.concatenate([cos32] * 4, 0)).astype(bf)
    sin128 = np.ascontiguousarray(np.concatenate([sin32] * 4, 0)).astype(bf)
    wkv_bw = (wkv_b * kv_norm_w[:, None]).astype(np.float32)
    xtf = np.ascontiguousarray(x.reshape(B * T, D).T)
    xt8 = xtf.astype(f8)
    wa = np.ascontiguousarray(wkv_a).astype(bf)
    wo_c = np.ascontiguousarray(wo).astype(bf)
    wq_r = wq.reshape(D, H, QKD)

    kk = np.arange(128)[:, None]
    qq = np.arange(BLK)[None, :]
    msk = np.concatenate(
        [(qq >= kk + j * 128).astype(np.float32) for j in range(4)], axis=1
    ).astype(bf)

    in_maps = []
    for c in range(NCORES):
        h0 = HPC * c
        w1f = np.concatenate(
            [
                wq_r[:, h0, :NOPE],
                wq_r[:, h0 + 1, :NOPE],
                wq_r[:, h0, NOPE:],
                wq_r[:, h0 + 1, NOPE:],
            ],
            axis=1,
        ) * (scale * S)
        # permute to rows (kp, p), cols (mt, two, mw): each DoubleRow
        # stationary slice [p, two, mw] must be contiguous in SBUF
        w1p = w1f.reshape(8, 2, 128, 3, 128).transpose(0, 2, 3, 1, 4)
        w1 = np.ascontiguousarray(w1p.reshape(1024, 768)).astype(f8)
        xslice = np.ascontiguousarray(xtf[:, c * BLK : (c + 1) * BLK]).astype(bf)
        xs01 = np.ascontiguousarray(xtf[:, : 2 * BLK]).astype(bf)
        wb = np.ascontiguousarray(
            wkv_bw[:, h0 * (NOPE + VD) : (h0 + 2) * (NOPE + VD)]
        ).astype(bf)
        in_maps.append(
            {
                "xt8": xt8,
                "xs": xslice,
                "xs01": xs01,
                "w1": w1,
                "wa": wa,
                "wb": wb,
                "wo": wo_c,
                "cos": cos128,
                "sin": sin128,
                "msk": msk,
            }
        )
    return in_maps


_NC = None


def kernel(x, wq, wkv_a, wkv_b, wo, kv_norm_w, _trace=False):
    global _NC
    if _NC is None:
        _NC = build_program()
    in_maps = host_prep(
        np.asarray(x, np.float32),
        np.asarray(wq, np.float32),
        np.asarray(wkv_a, np.float32),
        np.asarray(wkv_b, np.float32),
        np.asarray(wo, np.float32),
        np.asarray(kv_norm_w, np.float32),
    )
    res = run_bass_kernel_spmd(_NC, in_maps, list(range(NCORES)), trace=_trace)
    out = np.empty((B, T, D), np.float32)
    cw = T // NCORES
    for c in range(NCORES):
        oc = res.results[c]["out"].astype(np.float32)  # (B, 256, D)
        for b in range(B):
            out[b, c * cw : (c + 1) * cw, :] = oc[b]
    kernel.last_results = res
    return out


# revision 39
# speedup vs baseline: 1.0666x; 1.0492x over previous
"""Multi-Head Latent Attention (MLA) forward on 8 Trainium2 NeuronCores.

Sharding: tensor-parallel over heads (16 heads -> 2 per core), with the
latent-kv encoding sharded by TOKENS:
  - each core computes the full 576-dim kvc for its own 512-token slice
    from a 2MB bf16 slice of x.T; the result is distributed through FIVE
    pipelined AllGathers (one per 128-column tile), and the kv
    up-projection consumes each latent tile as it arrives,
  - q projections run as fp8e4 DoubleRow matmuls (K=256 per pass, 2x PE
    throughput) on an fp8 copy of x.T; the fp8 weight scale S is undone
    for free by the softmax exp's scale argument (exp(st/S)),
  - the latent path stays bf16 through the up-projection; k (nope+rope)
    and q are then quantized to fp8 so each score tile is a single
    DoubleRow matmul: k-tile 0 = k_nope, k-tile 1 = k_rope zero-padded,
    with per-head zero masks selecting that head's rope rows,
  - causal attention in transposed-score layout; exp/mask/denominator/
    attention-v work only on the valid lower-triangular column ranges of
    diagonal tiles; the denominator accumulates on the Vector engine in
    f32r so neither the PE nor the slow GpSimd gates the chain,
  - per-batch AllToAll exchanges head outputs; wo loads mid-flight and the
    last AllToAll is drained with a single rearranged DMA.
Output slices are disjoint; the host just concatenates them.
"""
import sys

if "/opt/trn_rl_repo" not in sys.path:
    sys.path.insert(0, "/opt/trn_rl_repo")

import numpy as np
import ml_dtypes
import concourse.bacc as bacc
import concourse.mybir as mybir
from concourse import tile
from concourse.bass_utils import run_bass_kernel_spmd

H, NOPE, ROPE, VD, KVR, QKD = 16, 128, 64, 128, 512, 192
B, T, D = 2, 2048, 2048
NCORES, HPC, BLK = 8, 2, 512
KVC = KVR + ROPE  # 576 latent+rope columns
S = 128.0  # fp8 weight scale, undone inside the softmax exp
f32 = mybir.dt.float32
f32r = mybir.dt.float32r
bf16 = mybir.dt.bfloat16
fp8 = mybir.dt.float8e4
DR = mybir.MatmulPerfMode.DoubleRow
EXP = mybir.ActivationFunctionType.Exp
LN = mybir.ActivationFunctionType.Ln
SQUARE = mybir.ActivationFunctionType.Square


def _patch_act_tables():
    """Make the act-table-load pass serve Exp/Ln/Square from the one set that
    contains them all (natural_log_exp_and_others), so interleaved activations
    don't thrash table loads. Indices into act_info.json must be preserved, so
    the shadowing single-function sets are emptied in place, not removed."""
    import concourse.bacc as _bacc

    orig = _bacc.get_activation_tables
    if getattr(_bacc, "_mla_act_patch", False):
        return
    _bacc._mla_act_patch = True

    def patched(arch):
        d = dict(orig(arch))
        if "natural_log_exp_and_others" in d:
            for name in ("exp_and_others", "natural_log", "exp_and_friends"):
                if name in d:
                    d[name] = set()
        return d

    _bacc.get_activation_tables = patched


def build_program():
    _patch_act_tables()
    nc = bacc.Bacc("TRN2", target_bir_lowering=False, debug=False, num_devices=NCORES)
    xt8_d = nc.dram_tensor("xt8", [D, B * T], fp8, kind="ExternalInput")
    xs_d = nc.dram_tensor("xs", [D, BLK], bf16, kind="ExternalInput")
    xs01_d = nc.dram_tensor("xs01", [D, 2 * BLK], bf16, kind="ExternalInput")
    # w1 pre-permuted on host: rows (kp, p), cols (mt, two, mw) so each
    # DoubleRow stationary slice [128, 2, 128] is contiguous (ISA requirement)
    w1_d = nc.dram_tensor("w1", [D // 2, 768], fp8, kind="ExternalInput")
    wa_d = nc.dram_tensor("wa", [D, KVC], bf16, kind="ExternalInput")
    wb_d = nc.dram_tensor("wb", [KVR, HPC * (NOPE + VD)], bf16, kind="ExternalInput")
    wo_d = nc.dram_tensor("wo", [H * VD, D], bf16, kind="ExternalInput")
    cos_d = nc.dram_tensor("cos", [128, T], bf16, kind="ExternalInput")
    sin_d = nc.dram_tensor("sin", [128, T], bf16, kind="ExternalInput")
    msk_d = nc.dram_tensor("msk", [128, 4 * BLK], bf16, kind="ExternalInput")
    out_d = nc.dram_tensor("out", [B, T // NCORES, D], bf16, kind="ExternalOutput")

    RG = [list(range(NCORES))]

    with tile.TileContext(nc) as tc:
        with (
            tc.tile_pool(name="dram", bufs=1, space="DRAM") as dram,
            tc.tile_pool(name="const", bufs=1) as const,
            tc.tile_pool(name="wpool", bufs=1) as wpool,
            tc.tile_pool(name="kvpool", bufs=1) as kvpool,
            tc.tile_pool(name="work", bufs=1) as work,
            tc.tile_pool(name="wop", bufs=1) as wop,
            tc.tile_pool(name="ps", bufs=1, space="PSUM") as ps,
        ):
            y_in = [
                dram.tile([NCORES, HPC * VD, 256], bf16, name=f"y_in{b}")
                for b in range(B)
            ]
            y_out = [
                dram.tile([NCORES, HPC * VD, 256], bf16, name=f"y_out{b}")
                for b in range(B)
            ]
            MTS = [128, 128, 128, 128, 64]
            ag_in = dram.tile([KVC, BLK], bf16, name="ag_in")
            ag_out = dram.tile(
                [NCORES * KVC, BLK], bf16, name="ag_out", addr_space="Shared"
            )

            agd_in = dram.tile([1, 64], bf16, name="agd_in")
            agd_out = dram.tile([8, 64], bf16, name="agd_out", addr_space="Shared")

            ones_f = const.tile([128, 1], f32, tag="ones_f")
            nc.gpsimd.memset(ones_f[:], 1.0)
            ones_r = const.tile([128, 1], f32r, tag="ones_r")
            nc.vector.tensor_copy(ones_r[:], ones_f[:])
            ones_b = const.tile([128, 1], bf16, tag="ones_b")
            nc.vector.tensor_copy(ones_b[:], ones_f[:])
            onesrow_f = const.tile([1, 128], f32, tag="onesrow_f")
            nc.gpsimd.memset(onesrow_f[:], 1.0)
            onesrow_b = const.tile([1, 128], bf16, tag="onesrow_b")
            nc.vector.tensor_copy(onesrow_b[:], onesrow_f[:])
            eps = const.tile([1, 1], f32, tag="eps")
            nc.gpsimd.memset(eps[:], 1e-6)

            dummy_sb = const.tile([1, 64], bf16, tag="dummy")
            nc.gpsimd.memset(dummy_sb[:], 0.0)
            nc.sync.dma_start(agd_in[:], dummy_sb[:])
            nc.gpsimd.collective_compute(
                "AllGather",
                mybir.AluOpType.bypass,
                replica_groups=RG,
                ins=[agd_in.opt()],
                outs=[agd_out.opt()],
            )

            # xs shares its SBUF slot with the later a2a staging tiles, and wa
            # with the big wo tile: both are dead before their partner loads.
            # Loads split into kc quarters so the kvc matmuls start ~4us in.
            xs_sb = wop.tile([128, 16, BLK], bf16, tag="a2a", bufs=1, name="xs")
            wa_sb = wpool.tile([128, 16, KVC], bf16, tag="wavo", bufs=1, name="wa")
            for g in range(4):
                kcs = slice(g * 4, g * 4 + 4)
                rows = slice(g * 512, (g + 1) * 512)
                nc.sync.dma_start(
                    xs_sb[:, kcs, :],
                    xs_d[rows, :].rearrange("(kc p) t -> p kc t", p=128),
                )
                nc.sync.dma_start(
                    wa_sb[:, kcs, :],
                    wa_d[rows, :].rearrange("(kc p) m -> p kc m", p=128),
                )
            w1_sb = wpool.tile([128, 8, 3, 2, 128], fp8, tag="w1")
            nc.sync.dma_start(
                w1_sb[:],
                w1_d[:].rearrange(
                    "(kp p) (mt two mw) -> p kp mt two mw", p=128, mt=3, two=2
                ),
            )
            wb_sb = wpool.tile([128, 4, 512], bf16, tag="wb")
            nc.sync.dma_start(wb_sb[:], wb_d[:].rearrange("(kc p) m -> p kc m", p=128))
            cs_sb = wpool.tile([128, T], bf16, tag="cs")
            nc.sync.dma_start(cs_sb[:], cos_d[:])
            sn_sb = wpool.tile([128, T], bf16, tag="sn")
            nc.sync.dma_start(sn_sb[:], sin_d[:])
            msk_sb = wpool.tile([128, 4 * BLK], bf16, tag="msk")

            def load_msk():
                nc.sync.dma_start(msk_sb[:], msk_d[:])

            wo_box = {}

            def load_wo():
                wo_box["wo"] = wpool.tile(
                    [128, 16, D], bf16, tag="wavo", bufs=1, name="wo"
                )
                nc.sync.dma_start(
                    wo_box["wo"][:], wo_d[:].rearrange("(kc p) m -> p kc m", p=128)
                )

            def kvc_block():
                """Full 576-col kvc for this core's 512-token slice + one AG."""
                for mt in range(5):
                    m0 = mt * 128
                    mw = MTS[mt]
                    pc = ps.tile([128, BLK], f32, tag="proj", bufs=4, name="pc")
                    for kc in range(16):
                        nc.tensor.matmul(
                            pc[:mw, :],
                            wa_sb[:, kc, m0 : m0 + mw],
                            xs_sb[:, kc, :],
                            start=(kc == 0),
                            stop=(kc == 15),
                        )
                    kvcm = work.tile([128, BLK], bf16, tag="kvcm", bufs=2)
                    nc.vector.tensor_copy(kvcm[:mw, :], pc[:mw, :])
                    nc.sync.dma_start(ag_in[m0 : m0 + mw, :], kvcm[:mw, :])
                nc.gpsimd.collective_compute(
                    "AllGather",
                    mybir.AluOpType.bypass,
                    replica_groups=RG,
                    ins=[ag_in.opt()],
                    outs=[ag_out.opt()],
                )

            def rep_kvc(r):
                """Recompute chunk (0, r)'s 576-col kvc locally (every core) so
                batch-0 chunks 0/1 attention runs before the AG lands."""
                pk = [
                    ps.tile([128, BLK], f32, tag="proj", bufs=4, name="rkp")
                    for _ in range(4)
                ]
                pr = ps.tile([128, BLK], f32, tag="xps", bufs=1, name="rkr")
                for kc in range(16):
                    xr = work.tile([128, BLK], bf16, tag="xrk", bufs=4)
                    nc.sync.dma_start(
                        xr[:],
                        xs01_d[kc * 128 : (kc + 1) * 128, r * BLK : (r + 1) * BLK],
                    )
                    for mt in range(4):
                        nc.tensor.matmul(
                            pk[mt][:],
                            wa_sb[:, kc, mt * 128 : (mt + 1) * 128],
                            xr[:],
                            start=(kc == 0),
                            stop=(kc == 15),
                        )
                    nc.tensor.matmul(
                        pr[:64, :],
                        wa_sb[:, kc, 512:],
                        xr[:],
                        start=(kc == 0),
                        stop=(kc == 15),
                    )
                latks = []
                for mt in range(4):
                    latk = work.tile([128, BLK], bf16, tag="latk", bufs=4)
                    nc.vector.tensor_copy(latk[:], pk[mt][:])
                    latks.append(latk)
                kraw = work.tile([ROPE, BLK], bf16, tag="kraw", bufs=2)
                nc.vector.tensor_copy(kraw[:], pr[:64, :])
                return latks, kraw

            # per-batch kv staging in fp8 score layout: kn8[h] holds k-tile 0
            # (k_nope) and k-tile 1 (k_rope in that head's partition half,
            # zeros elsewhere); vnat stays bf16 for the attention-v matmul
            def alloc_kv(b):
                kn8 = [
                    kvpool.tile(
                        [128, 16, 2, 128], fp8, tag="kn8", bufs=2, name=f"kn8{b}_{h}"
                    )
                    for h in range(2)
                ]
                vnat = [
                    kvpool.tile(
                        [128, 16, VD], bf16, tag="vnat", bufs=2, name=f"vnat{b}_{h}"
                    )
                    for h in range(2)
                ]
                if b == 0:
                    # the zero halves are never overwritten; batch 1 reuses
                    # these buffers with the zeros intact
                    nc.gpsimd.memset(kn8[0][64:128, :, 1, :], 0.0)
                    nc.gpsimd.memset(kn8[1][0:64, :, 1, :], 0.0)
                return kn8, vnat

            def stage_a1(b, qc):
                """fp8 x chunk load + DoubleRow q projection + q rope.

                qf8[h, 0] = S*q_nope(h); qf8[*, 1] = the roped 128-row chunk
                (h0 rope rows 0:64, h1 rows 64:128) -- written identically for
                both heads, the stationary zero rows select the right half."""
                row0 = b * T + qc * BLK
                tok = slice(qc * BLK, (qc + 1) * BLK)

                xt8c = work.tile([128, 8, 2, BLK], fp8, tag="xt8c", bufs=2)
                nc.sync.dma_start(
                    xt8c[:],
                    xt8_d[:, row0 : row0 + BLK].rearrange(
                        "(kp two p) t -> p kp two t", p=128, two=2
                    ),
                )

                qf8 = work.tile([128, 2, 2, BLK], fp8, tag="qf8", bufs=8)
                rot = work.tile([128, BLK], bf16, tag="rot", bufs=2)
                for mc in range(3):
                    pq = ps.tile([128, BLK], f32, tag="proj", bufs=4, name="pq")
                    for kp in range(8):
                        nc.tensor.matmul(
                            pq[:],
                            w1_sb[:, kp, mc, :, :],
                            xt8c[:, kp, :, :],
                            start=(kp == 0),
                            stop=(kp == 7),
                            perf_mode=DR,
                        )
                    if mc < 2:
                        nc.vector.tensor_copy(qf8[:, mc, 0, :], pq[:])
                    else:
                        for hh in range(2):
                            r0 = hh * 64
                            nc.vector.tensor_scalar_mul(
                                rot[r0 : r0 + 32, :], pq[r0 + 32 : r0 + 64, :], -1.0
                            )
                            nc.vector.tensor_copy(
                                rot[r0 + 32 : r0 + 64, :], pq[r0 : r0 + 32, :]
                            )
                        qre = work.tile([128, BLK], bf16, tag="qre", bufs=2)
                        nc.vector.tensor_mul(out=qre[:], in0=pq[:], in1=cs_sb[:, tok])
                        nc.vector.tensor_mul(out=rot[:], in0=rot[:], in1=sn_sb[:, tok])
                        nc.vector.tensor_add(out=qre[:], in0=qre[:], in1=rot[:])
                        nc.vector.tensor_copy(qf8[:, 0, 1, :], qre[:])
                        nc.scalar.activation(
                            qf8[:, 1, 1, :], qre[:],
                            mybir.ActivationFunctionType.Copy,
                        )
                return qf8

            def stage_a2(b, qc, kv, local=None):
                """post-AG (or from local replication): rms norm, kv
                up-projection, k rope; k stored fp8 in score layout."""
                kn8, vnat = kv
                src = b * 4 + qc
                tok = slice(qc * BLK, (qc + 1) * BLK)

                ssq = ps.tile([1, BLK], f32, tag="xps", bufs=1, name="ssq")
                pkv = [
                    ps.tile([128, BLK], f32, tag="proj", bufs=4, name="pkvp")
                    for _ in range(4)
                ]
                for kc in range(4):
                    if local is None:
                        latk = work.tile([128, BLK], bf16, tag="latk", bufs=4)
                        nc.scalar.dma_start(
                            latk[:],
                            ag_out[src * KVC + kc * 128 : src * KVC + (kc + 1) * 128, :],
                        )
                    else:
                        latk = local[0][kc]
                    sqc = work.tile([128, BLK], bf16, tag="sqc", bufs=2)
                    nc.scalar.activation(sqc[:], latk[:], SQUARE)
                    nc.tensor.matmul(
                        ssq[:], ones_b[:], sqc[:], start=(kc == 0), stop=(kc == 3)
                    )
                    for mc in range(4):  # [h0 nope, h0 v, h1 nope, h1 v]
                        nc.tensor.matmul(
                            pkv[mc][:],
                            wb_sb[:, kc, mc * 128 : (mc + 1) * 128],
                            latk[:],
                            start=(kc == 0),
                            stop=(kc == 3),
                        )
                if local is None:
                    kraw = work.tile([ROPE, BLK], bf16, tag="kraw", bufs=2)
                    nc.scalar.dma_start(
                        kraw[:], ag_out[src * KVC + KVR : (src + 1) * KVC, :]
                    )
                else:
                    kraw = local[1]

                # rms scale: 1/sqrt(ssq/512+eps) = exp(-0.5*ln(.))
                lnrow = work.tile([1, BLK], f32, tag="lnrow", bufs=1)
                nc.scalar.activation(lnrow[:], ssq[:], LN, bias=eps[:], scale=1.0 / KVR)
                invrow = work.tile([1, BLK], bf16, tag="invrow", bufs=1)
                nc.scalar.activation(invrow[:], lnrow[:], EXP, scale=-0.5)
                invbc_ps = ps.tile([128, BLK], f32, tag="xps", bufs=1, name="invbc_ps")
                nc.tensor.matmul(invbc_ps[:], onesrow_b[:], invrow[:])
                invbc = work.tile([128, BLK], bf16, tag="invbc", bufs=2)
                nc.vector.tensor_copy(invbc[:], invbc_ps[:])

                # k rope from gathered raw rows -> both heads' fp8 tiles
                rot = work.tile([128, BLK], bf16, tag="rot", bufs=2)
                kr = work.tile([ROPE, BLK], bf16, tag="krw", bufs=2)
                nc.vector.tensor_scalar_mul(rot[0:32, :], kraw[32:64, :], -1.0)
                nc.vector.tensor_copy(rot[32:64, :], kraw[0:32, :])
                nc.vector.tensor_mul(out=kr[:], in0=kraw[:], in1=cs_sb[0:64, tok])
                nc.vector.tensor_mul(
                    out=rot[0:64, :], in0=rot[0:64, :], in1=sn_sb[0:64, tok]
                )
                nc.vector.tensor_add(out=kr[:], in0=kr[:], in1=rot[0:64, :])
                kt4 = slice(qc * 4, qc * 4 + 4)
                nc.vector.tensor_copy(kn8[0][0:64, kt4, 1, :], kr[:])
                nc.scalar.activation(
                    kn8[1][64:128, kt4, 1, :], kr[:],
                    mybir.ActivationFunctionType.Copy,
                )

                # normalize + store k_nope (fp8) and v (bf16, natural layout)
                for mc in range(4):
                    h = mc // 2
                    if mc % 2 == 0:
                        nc.vector.tensor_mul(
                            out=kn8[h][:, kt4, 0, :], in0=pkv[mc][:], in1=invbc[:]
                        )
                    else:
                        vuT = work.tile([128, BLK], bf16, tag="vuT", bufs=2)
                        nc.vector.tensor_mul(out=vuT[:], in0=pkv[mc][:], in1=invbc[:])
                        nc.sync.dma_start_transpose(
                            vnat[h][:, qc * 4 : qc * 4 + 4, :], vuT[:]
                        )

            def stage_bh(b, qc, h, qf8, kv):
                """causal attention for one q-chunk, one head.

                One fp8 DoubleRow matmul per score tile (k_nope + padded
                k_rope contract together). exp/mask/acc/yacc trim to the
                valid diagonal column ranges. Lookahead-2; denominator on
                Vector in f32r."""
                kn8, vnat = kv
                n_kt = 4 * (qc + 1)
                yacc = ps.tile([VD, BLK], f32, tag="yacc", bufs=1)
                acc = work.tile([128, BLK], f32r, tag="acc", bufs=2)
                accg = work.tile([128, BLK], f32r, tag="accg", bufs=2)
                pend = []
                for kt in range(n_kt):
                    j = kt - 4 * qc
                    c0 = max(j, 0) * 128  # first valid q column
                    qs = slice(c0, BLK)
                    st = ps.tile([128, BLK], f32, tag="st", bufs=2)
                    nc.tensor.matmul(
                        st[:], kn8[h][:, kt, :, :], qf8[:, h, :, :],
                        start=True, stop=True, perf_mode=DR,
                    )
                    if len(pend) == 2:
                        pe_est, pk, pqs = pend.pop(0)
                        nc.tensor.matmul(
                            yacc[:, pqs], vnat[h][:, pk, :], pe_est[:, pqs],
                            start=(pk == 0), stop=False,
                        )
                    est = work.tile([128, BLK], bf16, tag="est", bufs=4)
                    nc.scalar.activation(est[:, qs], st[:, qs], EXP, scale=1.0 / S)
                    if j >= 0:
                        nc.vector.tensor_mul(
                            out=est[:, qs], in0=est[:, qs],
                            in1=msk_sb[:, j * BLK + c0 : (j + 1) * BLK],
                        )
                    if kt % 3 == 2:  # every third tile accumulates on gpsimd
                        if kt == 2:
                            nc.gpsimd.tensor_copy(accg[:, qs], est[:, qs])
                            if c0:
                                nc.gpsimd.memset(accg[:, :c0].bitcast(f32), 0.0)
                        else:
                            nc.gpsimd.tensor_add(
                                out=accg[:, qs], in0=accg[:, qs], in1=est[:, qs]
                            )
                    elif kt == 0:
                        nc.vector.tensor_copy(acc[:], est[:])
                    else:
                        nc.vector.tensor_add(
                            out=acc[:, qs], in0=acc[:, qs], in1=est[:, qs]
                        )
                    pend.append((est, kt, qs))
                for pe_est, pk, pqs in pend:
                    nc.tensor.matmul(
                        yacc[:, pqs], vnat[h][:, pk, :], pe_est[:, pqs],
                        start=(pk == 0), stop=(pk == n_kt - 1),
                    )

                sums = ps.tile([1, BLK], f32, tag="xps", bufs=1, name="sums")
                nc.tensor.matmul(sums[:], ones_r[:], acc[:], start=True, stop=False)
                nc.tensor.matmul(sums[:], ones_r[:], accg[:], start=False, stop=True)
                lnr = work.tile([1, BLK], f32, tag="lnrow", bufs=1)
                nc.scalar.activation(lnr[:], sums[:], LN)
                sinvrow = work.tile([1, BLK], bf16, tag="invrow", bufs=1)
                nc.scalar.activation(sinvrow[:], lnr[:], EXP, scale=-1.0)
                sbc_ps = ps.tile([128, BLK], f32, tag="xps", bufs=1, name="sbc_ps")
                nc.tensor.matmul(sbc_ps[:], onesrow_b[:], sinvrow[:])
                sinv = work.tile([128, BLK], bf16, tag="sinv", bufs=1)
                nc.vector.tensor_copy(sinv[:], sbc_ps[:])
                ysb = work.tile([VD, BLK], bf16, tag="ysb", bufs=1)
                nc.vector.tensor_mul(out=ysb[:], in0=yacc[:], in1=sinv[:])
                for jj in range(2):
                    nc.sync.dma_start(
                        y_in[b][qc * 2 + jj, h * VD : (h + 1) * VD, :],
                        ysb[:, jj * 256 : (jj + 1) * 256],
                    )

            def emit_a2a(b):
                nc.gpsimd.collective_compute(
                    "AllToAll",
                    mybir.AluOpType.bypass,
                    replica_groups=RG,
                    ins=[y_in[b].opt()],
                    outs=[y_out[b].opt()],
                )

            a2a_sb = {}

            def wo_load(b):
                a2a = wop.tile([128, 16, 256], bf16, tag="a2a", bufs=1, name=f"a2a{b}")
                nc.scalar.dma_start(
                    a2a[:], y_out[b][:].rearrange("c (h p) t -> p (c h) t", p=128)
                )
                a2a_sb[b] = a2a

            def wo_chains(b, chains):
                """wo projection chains for this batch's gathered token slice."""
                a2a = a2a_sb[b]
                for n, tt in chains:
                    pout = ps.tile([128, 512], f32, tag="proj", bufs=4, name="pout")
                    for kc in range(16):
                        nc.tensor.matmul(
                            pout[:],
                            a2a[:, kc, tt * 128 : (tt + 1) * 128],
                            wo_box["wo"][:, kc, n * 512 : (n + 1) * 512],
                            start=(kc == 0),
                            stop=(kc == 15),
                        )
                    osb = wop.tile([128, 512], bf16, tag="osb", bufs=2)
                    nc.scalar.activation(
                        osb[:], pout[:], mybir.ActivationFunctionType.Copy
                    )
                    nc.sync.dma_start(
                        out_d[
                            b, tt * 128 : (tt + 1) * 128, n * 512 : (n + 1) * 512
                        ],
                        osb[:],
                    )

            ALLC = [(n, tt) for n in range(4) for tt in range(2)]

            # ---- schedule (Tile reorders by dependencies; emission order
            # mostly matters for same-engine queue order) ----
            kv0 = alloc_kv(0)
            kv1 = alloc_kv(1)
            kvc_block()
            load_msk()
            q00 = stage_a1(0, 0)
            q01 = stage_a1(0, 1)
            loc0 = rep_kvc(0)
            stage_a2(0, 0, kv0, local=loc0)
            loc1 = rep_kvc(1)
            stage_a2(0, 1, kv0, local=loc1)
            stage_bh(0, 0, 0, q00, kv0)
            q02 = stage_a1(0, 2)
            stage_bh(0, 0, 1, q00, kv0)
            q03 = stage_a1(0, 3)
            stage_bh(0, 1, 0, q01, kv0)
            q10 = stage_a1(1, 0)
            stage_bh(0, 1, 1, q01, kv0)
            q11 = stage_a1(1, 1)
            q12 = stage_a1(1, 2)
            load_wo()
            q13 = stage_a1(1, 3)
            stage_a2(0, 2, kv0)
            stage_bh(0, 2, 0, q02, kv0)
            stage_bh(0, 2, 1, q02, kv0)
            stage_a2(0, 3, kv0)
            stage_bh(0, 3, 0, q03, kv0)
            stage_a2(1, 0, kv1)
            stage_bh(0, 3, 1, q03, kv0)
            emit_a2a(0)
            stage_a2(1, 1, kv1)
            stage_bh(1, 0, 0, q10, kv1)
            stage_bh(1, 0, 1, q10, kv1)
            stage_a2(1, 2, kv1)
            wo_load(0)
            wo_chains(0, ALLC[0:2])
            stage_bh(1, 1, 0, q11, kv1)
            stage_bh(1, 1, 1, q11, kv1)
            stage_a2(1, 3, kv1)
            wo_chains(0, ALLC[2:4])
            stage_bh(1, 2, 0, q12, kv1)
            stage_bh(1, 2, 1, q12, kv1)
            stage_bh(1, 3, 0, q13, kv1)
            stage_bh(1, 3, 1, q13, kv1)
            emit_a2a(1)
            wo_chains(0, ALLC[4:8])
            wo_load(1)
            wo_chains(1, ALLC)

    nc.compile()
    return nc


def host_prep(x, wq, wkv_a, wkv_b, wo, kv_norm_w):
    bf = ml_dtypes.bfloat16
    f8 = ml_dtypes.float8_e4m3fn
    scale = np.float32(QKD ** -0.5)
    inv = (1.0 / (10000.0 ** (np.arange(0, ROPE, 2, dtype=np.float32) / ROPE))).astype(
        np.float32
    )
    f = np.outer(np.arange(T, dtype=np.float32), inv)
    cos32 = np.cos(f).T.astype(np.float32)
    sin32 = np.sin(f).T.astype(np.float32)
    cos128 = np.ascontiguousarray(np.concatenate([cos32] * 4, 0)).astype(bf)
    sin128 = np.ascontiguousarray(np.concatenate([sin32] * 4, 0)).astype(bf)
    wkv_bw = (wkv_b * kv_norm_w[:, None]).astype(np.float32)
    xtf = np.ascontiguousarray(x.reshape(B * T, D).T)
    xt8 = xtf.astype(f8)
    wa = np.ascontiguousarray(wkv_a).astype(bf)
    wo_c = np.ascontiguousarray(wo).astype(bf)
    wq_r = wq.reshape(D, H, QKD)

    kk = np.arange(128)[:, None]
    qq = np.arange(BLK)[None, :]
    msk = np.concatenate(
        [(qq >= kk + j * 128).astype(np.float32) for j in range(4)], axis=1
    ).astype(bf)

    in_maps = []
    for c in range(NCORES):
        h0 = HPC * c
        w1f = np.concatenate(
            [
                wq_r[:, h0, :NOPE],
                wq_r[:, h0 + 1, :NOPE],
                wq_r[:, h0, NOPE:],
                wq_r[:, h0 + 1, NOPE:],
            ],
            axis=1,
        ) * (scale * S)
        # permute to rows (kp, p), cols (mt, two, mw): each DoubleRow
        # stationary slice [p, two, mw] must be contiguous in SBUF
        w1p = w1f.reshape(8, 2, 128, 3, 128).transpose(0, 2, 3, 1, 4)
        w1 = np.ascontiguousarray(w1p.reshape(1024, 768)).astype(f8)
        xslice = np.ascontiguousarray(xtf[:, c * BLK : (c + 1) * BLK]).astype(bf)
        xs01 = np.ascontiguousarray(xtf[:, : 2 * BLK]).astype(bf)
        wb = np.ascontiguousarray(
            wkv_bw[:, h0 * (NOPE + VD) : (h0 + 2) * (NOPE + VD)]
        ).astype(bf)
        in_maps.append(
            {
                "xt8": xt8,
                "xs": xslice,
                "xs01": xs01,
                "w1": w1,
                "wa": wa,
                "wb": wb,
                "wo": wo_c,
                "cos": cos128,
                "sin": sin128,
                "msk": msk,
            }
        )
    return in_maps


_NC = None


def kernel(x, wq, wkv_a, wkv_b, wo, kv_norm_w, _trace=False):
    global _NC
    if _NC is None:
        _NC = build_program()
    in_maps = host_prep(
        np.asarray(x, np.float32),
        np.asarray(wq, np.float32),
        np.asarray(wkv_a, np.float32),
        np.asarray(wkv_b, np.float32),
        np.asarray(wo, np.float32),
        np.asarray(kv_norm_w, np.float32),
    )
    res = run_bass_kernel_spmd(_NC, in_maps, list(range(NCORES)), trace=_trace)
    out = np.empty((B, T, D), np.float32)
    cw = T // NCORES
    for c in range(NCORES):
        oc = res.results[c]["out"].astype(np.float32)  # (B, 256, D)
        for b in range(B):
            out[b, c * cw : (c + 1) * cw, :] = oc[b]
    kernel.last_results = res
    return out


# revision 40
# speedup vs baseline: 1.0717x; 1.0048x over previous
"""Multi-Head Latent Attention (MLA) forward on 8 Trainium2 NeuronCores.

Sharding: tensor-parallel over heads (16 heads -> 2 per core), with the
latent-kv encoding sharded by TOKENS:
  - each core computes the full 576-dim kvc for its own 512-token slice
    from a 2MB bf16 slice of x.T; the result is distributed through FIVE
    pipelined AllGathers (one per 128-column tile), and the kv
    up-projection consumes each latent tile as it arrives,
  - q projections run as fp8e4 DoubleRow matmuls (K=256 per pass, 2x PE
    throughput) on an fp8 copy of x.T; the fp8 weight scale S is undone
    for free by the softmax exp's scale argument (exp(st/S)),
  - the latent path stays bf16 through the up-projection; k (nope+rope)
    and q are then quantized to fp8 so each score tile is a single
    DoubleRow matmul: k-tile 0 = k_nope, k-tile 1 = k_rope zero-padded,
    with per-head zero masks selecting that head's rope rows,
  - causal attention in transposed-score layout; exp/mask/denominator/
    attention-v work only on the valid lower-triangular column ranges of
    diagonal tiles; the denominator accumulates on the Vector engine in
    f32r so neither the PE nor the slow GpSimd gates the chain,
  - per-batch AllToAll exchanges head outputs; wo loads mid-flight and the
    last AllToAll is drained with a single rearranged DMA.
Output slices are disjoint; the host just concatenates them.
"""
import sys

if "/opt/trn_rl_repo" not in sys.path:
    sys.path.insert(0, "/opt/trn_rl_repo")

import numpy as np
import ml_dtypes
import concourse.bacc as bacc
import concourse.mybir as mybir
from concourse import tile
from concourse.bass_utils import run_bass_kernel_spmd

H, NOPE, ROPE, VD, KVR, QKD = 16, 128, 64, 128, 512, 192
B, T, D = 2, 2048, 2048
NCORES, HPC, BLK = 8, 2, 512
KVC = KVR + ROPE  # 576 latent+rope columns
S = 128.0  # fp8 weight scale, undone inside the softmax exp
f32 = mybir.dt.float32
f32r = mybir.dt.float32r
bf16 = mybir.dt.bfloat16
fp8 = mybir.dt.float8e4
DR = mybir.MatmulPerfMode.DoubleRow
EXP = mybir.ActivationFunctionType.Exp
LN = mybir.ActivationFunctionType.Ln
SQUARE = mybir.ActivationFunctionType.Square


def _patch_act_tables():
    """Make the act-table-load pass serve Exp/Ln/Square from the one set that
    contains them all (natural_log_exp_and_others), so interleaved activations
    don't thrash table loads. Indices into act_info.json must be preserved, so
    the shadowing single-function sets are emptied in place, not removed."""
    import concourse.bacc as _bacc

    orig = _bacc.get_activation_tables
    if getattr(_bacc, "_mla_act_patch", False):
        return
    _bacc._mla_act_patch = True

    def patched(arch):
        d = dict(orig(arch))
        if "natural_log_exp_and_others" in d:
            for name in ("exp_and_others", "natural_log", "exp_and_friends"):
                if name in d:
                    d[name] = set()
        return d

    _bacc.get_activation_tables = patched


def build_program():
    _patch_act_tables()
    nc = bacc.Bacc("TRN2", target_bir_lowering=False, debug=False, num_devices=NCORES)
    xt8_d = nc.dram_tensor("xt8", [D, B * T], fp8, kind="ExternalInput")
    xs_d = nc.dram_tensor("xs", [D, BLK], bf16, kind="ExternalInput")
    xs01_d = nc.dram_tensor("xs01", [D, 2 * BLK], bf16, kind="ExternalInput")
    # w1 pre-permuted on host: rows (kp, p), cols (mt, two, mw) so each
    # DoubleRow stationary slice [128, 2, 128] is contiguous (ISA requirement)
    w1_d = nc.dram_tensor("w1", [D // 2, 768], fp8, kind="ExternalInput")
    wa_d = nc.dram_tensor("wa", [D, KVC], bf16, kind="ExternalInput")
    wb_d = nc.dram_tensor("wb", [KVR, HPC * (NOPE + VD)], bf16, kind="ExternalInput")
    wo_d = nc.dram_tensor("wo", [H * VD, D], bf16, kind="ExternalInput")
    cos_d = nc.dram_tensor("cos", [128, T], bf16, kind="ExternalInput")
    sin_d = nc.dram_tensor("sin", [128, T], bf16, kind="ExternalInput")
    msk_d = nc.dram_tensor("msk", [128, 4 * BLK], bf16, kind="ExternalInput")
    out_d = nc.dram_tensor("out", [B, T // NCORES, D], bf16, kind="ExternalOutput")

    RG = [list(range(NCORES))]

    with tile.TileContext(nc) as tc:
        with (
            tc.tile_pool(name="dram", bufs=1, space="DRAM") as dram,
            tc.tile_pool(name="const", bufs=1) as const,
            tc.tile_pool(name="wpool", bufs=1) as wpool,
            tc.tile_pool(name="kvpool", bufs=1) as kvpool,
            tc.tile_pool(name="work", bufs=1) as work,
            tc.tile_pool(name="wop", bufs=1) as wop,
            tc.tile_pool(name="ps", bufs=1, space="PSUM") as ps,
        ):
            y_in = [
                dram.tile([NCORES, HPC * VD, 256], bf16, name=f"y_in{b}")
                for b in range(B)
            ]
            y_out = [
                dram.tile([NCORES, HPC * VD, 256], bf16, name=f"y_out{b}")
                for b in range(B)
            ]
            y_in1h = [
                dram.tile([NCORES, VD, 256], bf16, name=f"y_in1h{h}") for h in range(2)
            ]
            y_out1h = [
                dram.tile([NCORES, VD, 256], bf16, name=f"y_out1h{h}")
                for h in range(2)
            ]
            MTS = [128, 128, 128, 128, 64]
            ag_in = dram.tile([KVC, BLK], bf16, name="ag_in")
            ag_out = dram.tile(
                [NCORES * KVC, BLK], bf16, name="ag_out", addr_space="Shared"
            )

            agd_in = dram.tile([1, 64], bf16, name="agd_in")
            agd_out = dram.tile([8, 64], bf16, name="agd_out", addr_space="Shared")

            ones_f = const.tile([128, 1], f32, tag="ones_f")
            nc.gpsimd.memset(ones_f[:], 1.0)
            ones_r = const.tile([128, 1], f32r, tag="ones_r")
            nc.vector.tensor_copy(ones_r[:], ones_f[:])
            ones_b = const.tile([128, 1], bf16, tag="ones_b")
            nc.vector.tensor_copy(ones_b[:], ones_f[:])
            onesrow_f = const.tile([1, 128], f32, tag="onesrow_f")
            nc.gpsimd.memset(onesrow_f[:], 1.0)
            onesrow_b = const.tile([1, 128], bf16, tag="onesrow_b")
            nc.vector.tensor_copy(onesrow_b[:], onesrow_f[:])
            eps = const.tile([1, 1], f32, tag="eps")
            nc.gpsimd.memset(eps[:], 1e-6)

            dummy_sb = const.tile([1, 64], bf16, tag="dummy")
            nc.gpsimd.memset(dummy_sb[:], 0.0)
            nc.sync.dma_start(agd_in[:], dummy_sb[:])
            nc.gpsimd.collective_compute(
                "AllGather",
                mybir.AluOpType.bypass,
                replica_groups=RG,
                ins=[agd_in.opt()],
                outs=[agd_out.opt()],
            )

            # xs shares its SBUF slot with the later a2a staging tiles, and wa
            # with the big wo tile: both are dead before their partner loads.
            # Loads split into kc quarters so the kvc matmuls start ~4us in.
            xs_sb = wop.tile([128, 16, BLK], bf16, tag="a2a", bufs=1, name="xs")
            wa_sb = wpool.tile([128, 16, KVC], bf16, tag="wavo", bufs=1, name="wa")
            for g in range(4):
                kcs = slice(g * 4, g * 4 + 4)
                rows = slice(g * 512, (g + 1) * 512)
                nc.sync.dma_start(
                    xs_sb[:, kcs, :],
                    xs_d[rows, :].rearrange("(kc p) t -> p kc t", p=128),
                )
                nc.sync.dma_start(
                    wa_sb[:, kcs, :],
                    wa_d[rows, :].rearrange("(kc p) m -> p kc m", p=128),
                )
            w1_sb = wpool.tile([128, 8, 3, 2, 128], fp8, tag="w1")
            nc.sync.dma_start(
                w1_sb[:],
                w1_d[:].rearrange(
                    "(kp p) (mt two mw) -> p kp mt two mw", p=128, mt=3, two=2
                ),
            )
            wb_sb = wpool.tile([128, 4, 512], bf16, tag="wb")
            nc.sync.dma_start(wb_sb[:], wb_d[:].rearrange("(kc p) m -> p kc m", p=128))
            cs_sb = wpool.tile([128, T], bf16, tag="cs")
            nc.sync.dma_start(cs_sb[:], cos_d[:])
            sn_sb = wpool.tile([128, T], bf16, tag="sn")
            nc.sync.dma_start(sn_sb[:], sin_d[:])
            msk_sb = wpool.tile([128, 4 * BLK], bf16, tag="msk")

            def load_msk():
                nc.sync.dma_start(msk_sb[:], msk_d[:])

            wo_box = {}

            def load_wo():
                wo_box["wo"] = wpool.tile(
                    [128, 16, D], bf16, tag="wavo", bufs=1, name="wo"
                )
                nc.sync.dma_start(
                    wo_box["wo"][:], wo_d[:].rearrange("(kc p) m -> p kc m", p=128)
                )

            def kvc_block():
                """Full 576-col kvc for this core's 512-token slice + one AG."""
                for mt in range(5):
                    m0 = mt * 128
                    mw = MTS[mt]
                    pc = ps.tile([128, BLK], f32, tag="proj", bufs=4, name="pc")
                    for kc in range(16):
                        nc.tensor.matmul(
                            pc[:mw, :],
                            wa_sb[:, kc, m0 : m0 + mw],
                            xs_sb[:, kc, :],
                            start=(kc == 0),
                            stop=(kc == 15),
                        )
                    kvcm = work.tile([128, BLK], bf16, tag="kvcm", bufs=2)
                    nc.vector.tensor_copy(kvcm[:mw, :], pc[:mw, :])
                    nc.sync.dma_start(ag_in[m0 : m0 + mw, :], kvcm[:mw, :])
                nc.gpsimd.collective_compute(
                    "AllGather",
                    mybir.AluOpType.bypass,
                    replica_groups=RG,
                    ins=[ag_in.opt()],
                    outs=[ag_out.opt()],
                )

            def rep_kvc(r):
                """Recompute chunk (0, r)'s 576-col kvc locally (every core) so
                batch-0 chunks 0/1 attention runs before the AG lands."""
                pk = [
                    ps.tile([128, BLK], f32, tag="proj", bufs=4, name="rkp")
                    for _ in range(4)
                ]
                pr = ps.tile([128, BLK], f32, tag="xps", bufs=1, name="rkr")
                for kc in range(16):
                    xr = work.tile([128, BLK], bf16, tag="xrk", bufs=4)
                    nc.sync.dma_start(
                        xr[:],
                        xs01_d[kc * 128 : (kc + 1) * 128, r * BLK : (r + 1) * BLK],
                    )
                    for mt in range(4):
                        nc.tensor.matmul(
                            pk[mt][:],
                            wa_sb[:, kc, mt * 128 : (mt + 1) * 128],
                            xr[:],
                            start=(kc == 0),
                            stop=(kc == 15),
                        )
                    nc.tensor.matmul(
                        pr[:64, :],
                        wa_sb[:, kc, 512:],
                        xr[:],
                        start=(kc == 0),
                        stop=(kc == 15),
                    )
                latks = []
                for mt in range(4):
                    latk = work.tile([128, BLK], bf16, tag="latk", bufs=4)
                    nc.vector.tensor_copy(latk[:], pk[mt][:])
                    latks.append(latk)
                kraw = work.tile([ROPE, BLK], bf16, tag="kraw", bufs=2)
                nc.vector.tensor_copy(kraw[:], pr[:64, :])
                return latks, kraw

            # per-batch kv staging in fp8 score layout: kn8[h] holds k-tile 0
            # (k_nope) and k-tile 1 (k_rope in that head's partition half,
            # zeros elsewhere); vnat stays bf16 for the attention-v matmul
            def alloc_kv(b):
                kn8 = [
                    kvpool.tile(
                        [128, 16, 2, 128], fp8, tag="kn8", bufs=2, name=f"kn8{b}_{h}"
                    )
                    for h in range(2)
                ]
                vnat = [
                    kvpool.tile(
                        [128, 16, VD], bf16, tag="vnat", bufs=2, name=f"vnat{b}_{h}"
                    )
                    for h in range(2)
                ]
                if b == 0:
                    # the zero halves are never overwritten; batch 1 reuses
                    # these buffers with the zeros intact
                    nc.gpsimd.memset(kn8[0][64:128, :, 1, :], 0.0)
                    nc.gpsimd.memset(kn8[1][0:64, :, 1, :], 0.0)
                return kn8, vnat

            def stage_a1(b, qc):
                """fp8 x chunk load + DoubleRow q projection + q rope.

                qf8[h, 0] = S*q_nope(h); qf8[*, 1] = the roped 128-row chunk
                (h0 rope rows 0:64, h1 rows 64:128) -- written identically for
                both heads, the stationary zero rows select the right half."""
                row0 = b * T + qc * BLK
                tok = slice(qc * BLK, (qc + 1) * BLK)

                xt8c = work.tile([128, 8, 2, BLK], fp8, tag="xt8c", bufs=2)
                nc.sync.dma_start(
                    xt8c[:],
                    xt8_d[:, row0 : row0 + BLK].rearrange(
                        "(kp two p) t -> p kp two t", p=128, two=2
                    ),
                )

                qf8 = work.tile([128, 2, 2, BLK], fp8, tag="qf8", bufs=8)
                rot = work.tile([128, BLK], bf16, tag="rot", bufs=2)
                for mc in range(3):
                    pq = ps.tile([128, BLK], f32, tag="proj", bufs=4, name="pq")
                    for kp in range(8):
                        nc.tensor.matmul(
                            pq[:],
                            w1_sb[:, kp, mc, :, :],
                            xt8c[:, kp, :, :],
                            start=(kp == 0),
                            stop=(kp == 7),
                            perf_mode=DR,
                        )
                    if mc == 0:
                        nc.vector.tensor_copy(qf8[:, 0, 0, :], pq[:])
                    elif mc == 1:
                        nc.scalar.activation(
                            qf8[:, 1, 0, :], pq[:],
                            mybir.ActivationFunctionType.Copy,
                        )
                    else:
                        for hh in range(2):
                            r0 = hh * 64
                            nc.vector.tensor_scalar_mul(
                                rot[r0 : r0 + 32, :], pq[r0 + 32 : r0 + 64, :], -1.0
                            )
                            nc.vector.tensor_copy(
                                rot[r0 + 32 : r0 + 64, :], pq[r0 : r0 + 32, :]
                            )
                        qre = work.tile([128, BLK], bf16, tag="qre", bufs=2)
                        nc.vector.tensor_mul(out=qre[:], in0=pq[:], in1=cs_sb[:, tok])
                        nc.vector.tensor_mul(out=rot[:], in0=rot[:], in1=sn_sb[:, tok])
                        nc.vector.tensor_add(out=qre[:], in0=qre[:], in1=rot[:])
                        nc.vector.tensor_copy(qf8[:, 0, 1, :], qre[:])
                        nc.scalar.activation(
                            qf8[:, 1, 1, :], qre[:],
                            mybir.ActivationFunctionType.Copy,
                        )
                return qf8

            def stage_a2(b, qc, kv, local=None):
                """post-AG (or from local replication): rms norm, kv
                up-projection, k rope; k stored fp8 in score layout."""
                kn8, vnat = kv
                src = b * 4 + qc
                tok = slice(qc * BLK, (qc + 1) * BLK)

                ssq = ps.tile([1, BLK], f32, tag="xps", bufs=1, name="ssq")
                pkv = [
                    ps.tile([128, BLK], f32, tag="proj", bufs=4, name="pkvp")
                    for _ in range(4)
                ]
                for kc in range(4):
                    if local is None:
                        latk = work.tile([128, BLK], bf16, tag="latk", bufs=4)
                        nc.scalar.dma_start(
                            latk[:],
                            ag_out[src * KVC + kc * 128 : src * KVC + (kc + 1) * 128, :],
                        )
                    else:
                        latk = local[0][kc]
                    sqc = work.tile([128, BLK], bf16, tag="sqc", bufs=2)
                    nc.scalar.activation(sqc[:], latk[:], SQUARE)
                    nc.tensor.matmul(
                        ssq[:], ones_b[:], sqc[:], start=(kc == 0), stop=(kc == 3)
                    )
                    for mc in range(4):  # [h0 nope, h0 v, h1 nope, h1 v]
                        nc.tensor.matmul(
                            pkv[mc][:],
                            wb_sb[:, kc, mc * 128 : (mc + 1) * 128],
                            latk[:],
                            start=(kc == 0),
                            stop=(kc == 3),
                        )
                if local is None:
                    kraw = work.tile([ROPE, BLK], bf16, tag="kraw", bufs=2)
                    nc.scalar.dma_start(
                        kraw[:], ag_out[src * KVC + KVR : (src + 1) * KVC, :]
                    )
                else:
                    kraw = local[1]

                # rms scale: 1/sqrt(ssq/512+eps) = exp(-0.5*ln(.))
                lnrow = work.tile([1, BLK], f32, tag="lnrow", bufs=1)
                nc.scalar.activation(lnrow[:], ssq[:], LN, bias=eps[:], scale=1.0 / KVR)
                invrow = work.tile([1, BLK], bf16, tag="invrow", bufs=1)
                nc.scalar.activation(invrow[:], lnrow[:], EXP, scale=-0.5)
                invbc_ps = ps.tile([128, BLK], f32, tag="xps", bufs=1, name="invbc_ps")
                nc.tensor.matmul(invbc_ps[:], onesrow_b[:], invrow[:])
                invbc = work.tile([128, BLK], bf16, tag="invbc", bufs=2)
                nc.vector.tensor_copy(invbc[:], invbc_ps[:])

                # k rope from gathered raw rows -> both heads' fp8 tiles
                rot = work.tile([128, BLK], bf16, tag="rot", bufs=2)
                kr = work.tile([ROPE, BLK], bf16, tag="krw", bufs=2)
                nc.vector.tensor_scalar_mul(rot[0:32, :], kraw[32:64, :], -1.0)
                nc.vector.tensor_copy(rot[32:64, :], kraw[0:32, :])
                nc.vector.tensor_mul(out=kr[:], in0=kraw[:], in1=cs_sb[0:64, tok])
                nc.vector.tensor_mul(
                    out=rot[0:64, :], in0=rot[0:64, :], in1=sn_sb[0:64, tok]
                )
                nc.vector.tensor_add(out=kr[:], in0=kr[:], in1=rot[0:64, :])
                kt4 = slice(qc * 4, qc * 4 + 4)
                nc.vector.tensor_copy(kn8[0][0:64, kt4, 1, :], kr[:])
                nc.scalar.activation(
                    kn8[1][64:128, kt4, 1, :], kr[:],
                    mybir.ActivationFunctionType.Copy,
                )

                # normalize + store k_nope (fp8) and v (bf16, natural layout)
                for mc in range(4):
                    h = mc // 2
                    if mc % 2 == 0:
                        nc.vector.tensor_mul(
                            out=kn8[h][:, kt4, 0, :], in0=pkv[mc][:], in1=invbc[:]
                        )
                    else:
                        vuT = work.tile([128, BLK], bf16, tag="vuT", bufs=2)
                        nc.vector.tensor_mul(out=vuT[:], in0=pkv[mc][:], in1=invbc[:])
                        nc.sync.dma_start_transpose(
                            vnat[h][:, qc * 4 : qc * 4 + 4, :], vuT[:]
                        )

            def stage_bh(b, qc, h, qf8, kv):
                """causal attention for one q-chunk, one head.

                One fp8 DoubleRow matmul per score tile (k_nope + padded
                k_rope contract together). exp/mask/acc/yacc trim to the
                valid diagonal column ranges. Lookahead-2; denominator on
                Vector in f32r."""
                kn8, vnat = kv
                n_kt = 4 * (qc + 1)
                yacc = ps.tile([VD, BLK], f32, tag="yacc", bufs=1)
                acc = work.tile([128, BLK], f32r, tag="acc", bufs=2)
                accg = work.tile([128, BLK], f32r, tag="accg", bufs=2)
                pend = []
                for kt in range(n_kt):
                    j = kt - 4 * qc
                    c0 = max(j, 0) * 128  # first valid q column
                    qs = slice(c0, BLK)
                    st = ps.tile([128, BLK], f32, tag="st", bufs=2)
                    nc.tensor.matmul(
                        st[:], kn8[h][:, kt, :, :], qf8[:, h, :, :],
                        start=True, stop=True, perf_mode=DR,
                    )
                    if len(pend) == 2:
                        pe_est, pk, pqs = pend.pop(0)
                        nc.tensor.matmul(
                            yacc[:, pqs], vnat[h][:, pk, :], pe_est[:, pqs],
                            start=(pk == 0), stop=False,
                        )
                    est = work.tile([128, BLK], bf16, tag="est", bufs=5)
                    nc.scalar.activation(est[:, qs], st[:, qs], EXP, scale=1.0 / S)
                    if j >= 0:
                        nc.vector.tensor_mul(
                            out=est[:, qs], in0=est[:, qs],
                            in1=msk_sb[:, j * BLK + c0 : (j + 1) * BLK],
                        )
                    if kt % 3 == 2:  # every third tile accumulates on gpsimd
                        if kt == 2:
                            nc.gpsimd.tensor_copy(accg[:, qs], est[:, qs])
                            if c0:
                                nc.gpsimd.memset(accg[:, :c0].bitcast(f32), 0.0)
                        else:
                            nc.gpsimd.tensor_add(
                                out=accg[:, qs], in0=accg[:, qs], in1=est[:, qs]
                            )
                    elif kt == 0:
                        nc.vector.tensor_copy(acc[:], est[:])
                    else:
                        nc.vector.tensor_add(
                            out=acc[:, qs], in0=acc[:, qs], in1=est[:, qs]
                        )
                    pend.append((est, kt, qs))
                for pe_est, pk, pqs in pend:
                    nc.tensor.matmul(
                        yacc[:, pqs], vnat[h][:, pk, :], pe_est[:, pqs],
                        start=(pk == 0), stop=(pk == n_kt - 1),
                    )

                sums = ps.tile([1, BLK], f32, tag="xps", bufs=1, name="sums")
                nc.tensor.matmul(sums[:], ones_r[:], acc[:], start=True, stop=False)
                nc.tensor.matmul(sums[:], ones_r[:], accg[:], start=False, stop=True)
                lnr = work.tile([1, BLK], f32, tag="lnrow", bufs=1)
                nc.scalar.activation(lnr[:], sums[:], LN)
                sinvrow = work.tile([1, BLK], bf16, tag="invrow", bufs=1)
                nc.scalar.activation(sinvrow[:], lnr[:], EXP, scale=-1.0)
                sbc_ps = ps.tile([128, BLK], f32, tag="xps", bufs=1, name="sbc_ps")
                nc.tensor.matmul(sbc_ps[:], onesrow_b[:], sinvrow[:])
                sinv = work.tile([128, BLK], bf16, tag="sinv", bufs=1)
                nc.vector.tensor_copy(sinv[:], sbc_ps[:])
                ysb = work.tile([VD, BLK], bf16, tag="ysb", bufs=1)
                nc.vector.tensor_mul(out=ysb[:], in0=yacc[:], in1=sinv[:])
                for jj in range(2):
                    if b == 0:
                        nc.sync.dma_start(
                            y_in[0][qc * 2 + jj, h * VD : (h + 1) * VD, :],
                            ysb[:, jj * 256 : (jj + 1) * 256],
                        )
                    else:
                        nc.sync.dma_start(
                            y_in1h[h][qc * 2 + jj, :, :],
                            ysb[:, jj * 256 : (jj + 1) * 256],
                        )

            def emit_a2a(b):
                nc.gpsimd.collective_compute(
                    "AllToAll",
                    mybir.AluOpType.bypass,
                    replica_groups=RG,
                    ins=[y_in[b].opt()],
                    outs=[y_out[b].opt()],
                )

            def emit_a2a1h(h):
                nc.gpsimd.collective_compute(
                    "AllToAll",
                    mybir.AluOpType.bypass,
                    replica_groups=RG,
                    ins=[y_in1h[h].opt()],
                    outs=[y_out1h[h].opt()],
                )

            a2a_sb = {}

            def wo_load(b):
                a2a = wop.tile([128, 16, 256], bf16, tag="a2a", bufs=1, name=f"a2a{b}")
                if b == 0:
                    nc.scalar.dma_start(
                        a2a[:], y_out[0][:].rearrange("c (h p) t -> p (c h) t", p=128)
                    )
                else:
                    for h in range(2):
                        nc.scalar.dma_start(
                            a2a[:, h * 8 : h * 8 + 8, :],
                            y_out1h[h][:].rearrange("c p t -> p c t"),
                        )
                a2a_sb[b] = a2a

            def wo_chains(b, chains):
                """wo projection chains for this batch's gathered token slice."""
                a2a = a2a_sb[b]
                # batch 1's staging holds [h0 cores 0..7 | h1 cores 0..7]; wo
                # rows are (core, h) interleaved
                order = (
                    list(range(16))
                    if b == 0
                    else [h * 8 + c for h in range(2) for c in range(8)]
                )
                wokc = (
                    list(range(16))
                    if b == 0
                    else [c * 2 + h for h in range(2) for c in range(8)]
                )
                for n, tt in chains:
                    pout = ps.tile([128, 512], f32, tag="proj", bufs=4, name="pout")
                    for i in range(16):
                        nc.tensor.matmul(
                            pout[:],
                            a2a[:, order[i], tt * 128 : (tt + 1) * 128],
                            wo_box["wo"][:, wokc[i], n * 512 : (n + 1) * 512],
                            start=(i == 0),
                            stop=(i == 15),
                        )
                    osb = wop.tile([128, 512], bf16, tag="osb", bufs=2)
                    nc.scalar.activation(
                        osb[:], pout[:], mybir.ActivationFunctionType.Copy
                    )
                    nc.sync.dma_start(
                        out_d[
                            b, tt * 128 : (tt + 1) * 128, n * 512 : (n + 1) * 512
                        ],
                        osb[:],
                    )

            ALLC = [(n, tt) for n in range(4) for tt in range(2)]

            # ---- schedule (Tile reorders by dependencies; emission order
            # mostly matters for same-engine queue order) ----
            kv0 = alloc_kv(0)
            kv1 = alloc_kv(1)
            kvc_block()
            load_msk()
            q00 = stage_a1(0, 0)
            q01 = stage_a1(0, 1)
            loc0 = rep_kvc(0)
            stage_a2(0, 0, kv0, local=loc0)
            loc1 = rep_kvc(1)
            stage_a2(0, 1, kv0, local=loc1)
            stage_bh(0, 0, 0, q00, kv0)
            q02 = stage_a1(0, 2)
            stage_bh(0, 0, 1, q00, kv0)
            q03 = stage_a1(0, 3)
            stage_bh(0, 1, 0, q01, kv0)
            q10 = stage_a1(1, 0)
            stage_bh(0, 1, 1, q01, kv0)
            q11 = stage_a1(1, 1)
            q12 = stage_a1(1, 2)
            load_wo()
            q13 = stage_a1(1, 3)
            stage_a2(0, 2, kv0)
            stage_bh(0, 2, 0, q02, kv0)
            stage_bh(0, 2, 1, q02, kv0)
            stage_a2(0, 3, kv0)
            stage_bh(0, 3, 0, q03, kv0)
            stage_a2(1, 0, kv1)
            stage_bh(0, 3, 1, q03, kv0)
            emit_a2a(0)
            stage_a2(1, 1, kv1)
            stage_bh(1, 0, 0, q10, kv1)
            stage_bh(1, 0, 1, q10, kv1)
            stage_a2(1, 2, kv1)
            wo_load(0)
            wo_chains(0, ALLC[0:2])
            stage_bh(1, 1, 0, q11, kv1)
            stage_bh(1, 1, 1, q11, kv1)
            stage_a2(1, 3, kv1)
            wo_chains(0, ALLC[2:4])
            stage_bh(1, 2, 0, q12, kv1)
            stage_bh(1, 2, 1, q12, kv1)
            stage_bh(1, 3, 0, q13, kv1)
            emit_a2a1h(0)
            stage_bh(1, 3, 1, q13, kv1)
            emit_a2a1h(1)
            wo_chains(0, ALLC[4:8])
            wo_load(1)
            wo_chains(1, ALLC)

    nc.compile()
    return nc


def host_prep(x, wq, wkv_a, wkv_b, wo, kv_norm_w):
    bf = ml_dtypes.bfloat16
    f8 = ml_dtypes.float8_e4m3fn
    scale = np.float32(QKD ** -0.5)
    inv = (1.0 / (10000.0 ** (np.arange(0, ROPE, 2, dtype=np.float32) / ROPE))).astype(
        np.float32
    )
    f = np.outer(np.arange(T, dtype=np.float32), inv)
    cos32 = np.cos(f).T.astype(np.float32)
    sin32 = np.sin(f).T.astype(np.float32)
    cos128 = np.ascontiguousarray(np.concatenate([cos32] * 4, 0)).astype(bf)
    sin128 = np.ascontiguousarray(np.concatenate([sin32] * 4, 0)).astype(bf)
    wkv_bw = (wkv_b * kv_norm_w[:, None]).astype(np.float32)
    xtf = np.ascontiguousarray(x.reshape(B * T, D).T)
    xt8 = xtf.astype(f8)
    wa = np.ascontiguousarray(wkv_a).astype(bf)
    wo_c = np.ascontiguousarray(wo).astype(bf)
    wq_r = wq.reshape(D, H, QKD)

    kk = np.arange(128)[:, None]
    qq = np.arange(BLK)[None, :]
    msk = np.concatenate(
        [(qq >= kk + j * 128).astype(np.float32) for j in range(4)], axis=1
    ).astype(bf)

    in_maps = []
    for c in range(NCORES):
        h0 = HPC * c
        w1f = np.concatenate(
            [
                wq_r[:, h0, :NOPE],
                wq_r[:, h0 + 1, :NOPE],
                wq_r[:, h0, NOPE:],
                wq_r[:, h0 + 1, NOPE:],
            ],
            axis=1,
        ) * (scale * S)
        # permute to rows (kp, p), cols (mt, two, mw): each DoubleRow
        # stationary slice [p, two, mw] must be contiguous in SBUF
        w1p = w1f.reshape(8, 2, 128, 3, 128).transpose(0, 2, 3, 1, 4)
        w1 = np.ascontiguousarray(w1p.reshape(1024, 768)).astype(f8)
        xslice = np.ascontiguousarray(xtf[:, c * BLK : (c + 1) * BLK]).astype(bf)
        xs01 = np.ascontiguousarray(xtf[:, : 2 * BLK]).astype(bf)
        wb = np.ascontiguousarray(
            wkv_bw[:, h0 * (NOPE + VD) : (h0 + 2) * (NOPE + VD)]
        ).astype(bf)
        in_maps.append(
            {
                "xt8": xt8,
                "xs": xslice,
                "xs01": xs01,
                "w1": w1,
                "wa": wa,
                "wb": wb,
                "wo": wo_c,
                "cos": cos128,
                "sin": sin128,
                "msk": msk,
            }
        )
    return in_maps


_NC = None


def kernel(x, wq, wkv_a, wkv_b, wo, kv_norm_w, _trace=False):
    global _NC
    if _NC is None:
        _NC = build_program()
    in_maps = host_prep(
        np.asarray(x, np.float32),
        np.asarray(wq, np.float32),
        np.asarray(wkv_a, np.float32),
        np.asarray(wkv_b, np.float32),
        np.asarray(wo, np.float32),
        np.asarray(kv_norm_w, np.float32),
    )
    res = run_bass_kernel_spmd(_NC, in_maps, list(range(NCORES)), trace=_trace)
    out = np.empty((B, T, D), np.float32)
    cw = T // NCORES
    for c in range(NCORES):
        oc = res.results[c]["out"].astype(np.float32)  # (B, 256, D)
        for b in range(B):
            out[b, c * cw : (c + 1) * cw, :] = oc[b]
    kernel.last_results = res
    return out


# revision 41
# speedup vs baseline: 1.0767x; 1.0047x over previous
"""Multi-Head Latent Attention (MLA) forward on 8 Trainium2 NeuronCores.

Sharding: tensor-parallel over heads (16 heads -> 2 per core), with the
latent-kv encoding sharded by TOKENS:
  - each core computes the full 576-dim kvc for its own 512-token slice
    from a 2MB bf16 slice of x.T; the result is distributed through FIVE
    pipelined AllGathers (one per 128-column tile), and the kv
    up-projection consumes each latent tile as it arrives,
  - q projections run as fp8e4 DoubleRow matmuls (K=256 per pass, 2x PE
    throughput) on an fp8 copy of x.T; the fp8 weight scale S is undone
    for free by the softmax exp's scale argument (exp(st/S)),
  - the latent path stays bf16 through the up-projection; k (nope+rope)
    and q are then quantized to fp8 so each score tile is a single
    DoubleRow matmul: k-tile 0 = k_nope, k-tile 1 = k_rope zero-padded,
    with per-head zero masks selecting that head's rope rows,
  - causal attention in transposed-score layout; exp/mask/denominator/
    attention-v work only on the valid lower-triangular column ranges of
    diagonal tiles; the denominator accumulates on the Vector engine in
    f32r so neither the PE nor the slow GpSimd gates the chain,
  - per-batch AllToAll exchanges head outputs; wo loads mid-flight and the
    last AllToAll is drained with a single rearranged DMA.
Output slices are disjoint; the host just concatenates them.
"""
import sys

if "/opt/trn_rl_repo" not in sys.path:
    sys.path.insert(0, "/opt/trn_rl_repo")

import numpy as np
import ml_dtypes
import concourse.bacc as bacc
import concourse.mybir as mybir
from concourse import tile
from concourse.bass_utils import run_bass_kernel_spmd

H, NOPE, ROPE, VD, KVR, QKD = 16, 128, 64, 128, 512, 192
B, T, D = 2, 2048, 2048
NCORES, HPC, BLK = 8, 2, 512
KVC = KVR + ROPE  # 576 latent+rope columns
S = 128.0  # fp8 weight scale, undone inside the softmax exp
f32 = mybir.dt.float32
f32r = mybir.dt.float32r
bf16 = mybir.dt.bfloat16
fp8 = mybir.dt.float8e4
DR = mybir.MatmulPerfMode.DoubleRow
EXP = mybir.ActivationFunctionType.Exp
LN = mybir.ActivationFunctionType.Ln
SQUARE = mybir.ActivationFunctionType.Square


def _patch_act_tables():
    """Make the act-table-load pass serve Exp/Ln/Square from the one set that
    contains them all (natural_log_exp_and_others), so interleaved activations
    don't thrash table loads. Indices into act_info.json must be preserved, so
    the shadowing single-function sets are emptied in place, not removed."""
    import concourse.bacc as _bacc

    orig = _bacc.get_activation_tables
    if getattr(_bacc, "_mla_act_patch", False):
        return
    _bacc._mla_act_patch = True

    def patched(arch):
        d = dict(orig(arch))
        if "natural_log_exp_and_others" in d:
            for name in ("exp_and_others", "natural_log", "exp_and_friends"):
                if name in d:
                    d[name] = set()
        return d

    _bacc.get_activation_tables = patched


def build_program():
    _patch_act_tables()
    nc = bacc.Bacc("TRN2", target_bir_lowering=False, debug=False, num_devices=NCORES)
    xt8_d = nc.dram_tensor("xt8", [D, B * T], fp8, kind="ExternalInput")
    xs_d = nc.dram_tensor("xs", [D, BLK], bf16, kind="ExternalInput")
    xs01_d = nc.dram_tensor("xs01", [D, 2 * BLK], bf16, kind="ExternalInput")
    # w1 pre-permuted on host: rows (kp, p), cols (mt, two, mw) so each
    # DoubleRow stationary slice [128, 2, 128] is contiguous (ISA requirement)
    w1_d = nc.dram_tensor("w1", [D // 2, 768], fp8, kind="ExternalInput")
    wa_d = nc.dram_tensor("wa", [D, KVC], bf16, kind="ExternalInput")
    wb_d = nc.dram_tensor("wb", [KVR, HPC * (NOPE + VD)], bf16, kind="ExternalInput")
    wo_d = nc.dram_tensor("wo", [H * VD, D], bf16, kind="ExternalInput")
    cos_d = nc.dram_tensor("cos", [128, T], bf16, kind="ExternalInput")
    sin_d = nc.dram_tensor("sin", [128, T], bf16, kind="ExternalInput")
    msk_d = nc.dram_tensor("msk", [128, 4 * BLK], bf16, kind="ExternalInput")
    out_d = nc.dram_tensor("out", [B, T // NCORES, D], bf16, kind="ExternalOutput")

    RG = [list(range(NCORES))]

    with tile.TileContext(nc) as tc:
        with (
            tc.tile_pool(name="dram", bufs=1, space="DRAM") as dram,
            tc.tile_pool(name="const", bufs=1) as const,
            tc.tile_pool(name="wpool", bufs=1) as wpool,
            tc.tile_pool(name="kvpool", bufs=1) as kvpool,
            tc.tile_pool(name="work", bufs=1) as work,
            tc.tile_pool(name="wop", bufs=1) as wop,
            tc.tile_pool(name="ps", bufs=1, space="PSUM") as ps,
        ):
            y_in = [
                dram.tile([NCORES, HPC * VD, 256], bf16, name=f"y_in{b}")
                for b in range(B)
            ]
            y_out = [
                dram.tile([NCORES, HPC * VD, 256], bf16, name=f"y_out{b}")
                for b in range(B)
            ]
            y_in1h = [
                dram.tile([NCORES, VD, 256], bf16, name=f"y_in1h{h}") for h in range(2)
            ]
            y_out1h = [
                dram.tile([NCORES, VD, 256], bf16, name=f"y_out1h{h}")
                for h in range(2)
            ]
            MTS = [128, 128, 128, 128, 64]
            ag_in = dram.tile([KVC, BLK], bf16, name="ag_in")
            ag_out = dram.tile(
                [NCORES * KVC, BLK], bf16, name="ag_out", addr_space="Shared"
            )

            agd_in = dram.tile([1, 64], bf16, name="agd_in")
            agd_out = dram.tile([8, 64], bf16, name="agd_out", addr_space="Shared")

            ones_f = const.tile([128, 1], f32, tag="ones_f")
            nc.gpsimd.memset(ones_f[:], 1.0)
            ones_r = const.tile([128, 1], f32r, tag="ones_r")
            nc.vector.tensor_copy(ones_r[:], ones_f[:])
            ones_b = const.tile([128, 1], bf16, tag="ones_b")
            nc.vector.tensor_copy(ones_b[:], ones_f[:])
            onesrow_f = const.tile([1, 128], f32, tag="onesrow_f")
            nc.gpsimd.memset(onesrow_f[:], 1.0)
            onesrow_b = const.tile([1, 128], bf16, tag="onesrow_b")
            nc.vector.tensor_copy(onesrow_b[:], onesrow_f[:])
            eps = const.tile([1, 1], f32, tag="eps")
            nc.gpsimd.memset(eps[:], 1e-6)

            dummy_sb = const.tile([1, 64], bf16, tag="dummy")
            nc.gpsimd.memset(dummy_sb[:], 0.0)
            nc.sync.dma_start(agd_in[:], dummy_sb[:])
            nc.gpsimd.collective_compute(
                "AllGather",
                mybir.AluOpType.bypass,
                replica_groups=RG,
                ins=[agd_in.opt()],
                outs=[agd_out.opt()],
            )

            # xs shares its SBUF slot with the later a2a staging tiles, and wa
            # with the big wo tile: both are dead before their partner loads.
            # Loads split into kc quarters so the kvc matmuls start ~4us in.
            xs_sb = wop.tile([128, 16, BLK], bf16, tag="a2a", bufs=1, name="xs")
            wa_sb = wpool.tile([128, 16, KVC], bf16, tag="wavo", bufs=1, name="wa")
            for g in range(4):
                kcs = slice(g * 4, g * 4 + 4)
                rows = slice(g * 512, (g + 1) * 512)
                nc.sync.dma_start(
                    xs_sb[:, kcs, :],
                    xs_d[rows, :].rearrange("(kc p) t -> p kc t", p=128),
                )
                nc.sync.dma_start(
                    wa_sb[:, kcs, :],
                    wa_d[rows, :].rearrange("(kc p) m -> p kc m", p=128),
                )
            w1_sb = wpool.tile([128, 8, 3, 2, 128], fp8, tag="w1")
            nc.sync.dma_start(
                w1_sb[:],
                w1_d[:].rearrange(
                    "(kp p) (mt two mw) -> p kp mt two mw", p=128, mt=3, two=2
                ),
            )
            wb_sb = wpool.tile([128, 4, 512], bf16, tag="wb")
            nc.sync.dma_start(wb_sb[:], wb_d[:].rearrange("(kc p) m -> p kc m", p=128))
            cs_sb = wpool.tile([128, T], bf16, tag="cs")
            nc.sync.dma_start(cs_sb[:], cos_d[:])
            sn_sb = wpool.tile([128, T], bf16, tag="sn")
            nc.sync.dma_start(sn_sb[:], sin_d[:])
            msk_sb = wpool.tile([128, 4 * BLK], bf16, tag="msk")

            def load_msk():
                nc.sync.dma_start(msk_sb[:], msk_d[:])

            wo_box = {}

            def load_wo():
                wo_box["wo"] = wpool.tile(
                    [128, 16, D], bf16, tag="wavo", bufs=1, name="wo"
                )
                nc.sync.dma_start(
                    wo_box["wo"][:], wo_d[:].rearrange("(kc p) m -> p kc m", p=128)
                )

            def kvc_block():
                """Full 576-col kvc for this core's 512-token slice + one AG."""
                for mt in range(5):
                    m0 = mt * 128
                    mw = MTS[mt]
                    pc = ps.tile([128, BLK], f32, tag="proj", bufs=4, name="pc")
                    for kc in range(16):
                        nc.tensor.matmul(
                            pc[:mw, :],
                            wa_sb[:, kc, m0 : m0 + mw],
                            xs_sb[:, kc, :],
                            start=(kc == 0),
                            stop=(kc == 15),
                        )
                    kvcm = work.tile([128, BLK], bf16, tag="kvcm", bufs=2)
                    nc.vector.tensor_copy(kvcm[:mw, :], pc[:mw, :])
                    nc.sync.dma_start(ag_in[m0 : m0 + mw, :], kvcm[:mw, :])
                nc.gpsimd.collective_compute(
                    "AllGather",
                    mybir.AluOpType.bypass,
                    replica_groups=RG,
                    ins=[ag_in.opt()],
                    outs=[ag_out.opt()],
                )

            def rep_kvc(r):
                """Recompute chunk (0, r)'s 576-col kvc locally (every core) so
                batch-0 chunks 0/1 attention runs before the AG lands."""
                pk = [
                    ps.tile([128, BLK], f32, tag="proj", bufs=4, name="rkp")
                    for _ in range(4)
                ]
                pr = ps.tile([128, BLK], f32, tag="xps", bufs=1, name="rkr")
                for kc in range(16):
                    xr = work.tile([128, BLK], bf16, tag="xrk", bufs=4)
                    nc.sync.dma_start(
                        xr[:],
                        xs01_d[kc * 128 : (kc + 1) * 128, r * BLK : (r + 1) * BLK],
                    )
                    for mt in range(4):
                        nc.tensor.matmul(
                            pk[mt][:],
                            wa_sb[:, kc, mt * 128 : (mt + 1) * 128],
                            xr[:],
                            start=(kc == 0),
                            stop=(kc == 15),
                        )
                    nc.tensor.matmul(
                        pr[:64, :],
                        wa_sb[:, kc, 512:],
                        xr[:],
                        start=(kc == 0),
                        stop=(kc == 15),
                    )
                latks = []
                for mt in range(4):
                    latk = work.tile([128, BLK], bf16, tag="latk", bufs=4)
                    nc.vector.tensor_copy(latk[:], pk[mt][:])
                    latks.append(latk)
                kraw = work.tile([ROPE, BLK], bf16, tag="kraw", bufs=2)
                nc.vector.tensor_copy(kraw[:], pr[:64, :])
                return latks, kraw

            # per-batch kv staging in fp8 score layout: kn8[h] holds k-tile 0
            # (k_nope) and k-tile 1 (k_rope in that head's partition half,
            # zeros elsewhere); vnat stays bf16 for the attention-v matmul
            def alloc_kv(b):
                kn8 = [
                    kvpool.tile(
                        [128, 16, 2, 128], fp8, tag="kn8", bufs=2, name=f"kn8{b}_{h}"
                    )
                    for h in range(2)
                ]
                vnat = [
                    kvpool.tile(
                        [128, 16, VD], bf16, tag="vnat", bufs=2, name=f"vnat{b}_{h}"
                    )
                    for h in range(2)
                ]
                if b == 0:
                    # the zero halves are never overwritten; batch 1 reuses
                    # these buffers with the zeros intact
                    nc.gpsimd.memset(kn8[0][64:128, :, 1, :], 0.0)
                    nc.gpsimd.memset(kn8[1][0:64, :, 1, :], 0.0)
                return kn8, vnat

            def stage_a1(b, qc):
                """fp8 x chunk load + DoubleRow q projection + q rope.

                qf8[h, 0] = S*q_nope(h); qf8[*, 1] = the roped 128-row chunk
                (h0 rope rows 0:64, h1 rows 64:128) -- written identically for
                both heads, the stationary zero rows select the right half."""
                row0 = b * T + qc * BLK
                tok = slice(qc * BLK, (qc + 1) * BLK)

                xt8c = work.tile([128, 8, 2, BLK], fp8, tag="xt8c", bufs=2)
                nc.sync.dma_start(
                    xt8c[:],
                    xt8_d[:, row0 : row0 + BLK].rearrange(
                        "(kp two p) t -> p kp two t", p=128, two=2
                    ),
                )

                qf8 = work.tile([128, 2, 2, BLK], fp8, tag="qf8", bufs=8)
                rot = work.tile([128, BLK], bf16, tag="rot", bufs=2)
                for mc in range(3):
                    pq = ps.tile([128, BLK], f32, tag="proj", bufs=4, name="pq")
                    for kp in range(8):
                        nc.tensor.matmul(
                            pq[:],
                            w1_sb[:, kp, mc, :, :],
                            xt8c[:, kp, :, :],
                            start=(kp == 0),
                            stop=(kp == 7),
                            perf_mode=DR,
                        )
                    if mc == 0:
                        nc.vector.tensor_copy(qf8[:, 0, 0, :], pq[:])
                    elif mc == 1:
                        nc.scalar.activation(
                            qf8[:, 1, 0, :], pq[:],
                            mybir.ActivationFunctionType.Copy,
                        )
                    else:
                        for hh in range(2):
                            r0 = hh * 64
                            nc.vector.tensor_scalar_mul(
                                rot[r0 : r0 + 32, :], pq[r0 + 32 : r0 + 64, :], -1.0
                            )
                            nc.vector.tensor_copy(
                                rot[r0 + 32 : r0 + 64, :], pq[r0 : r0 + 32, :]
                            )
                        qre = work.tile([128, BLK], bf16, tag="qre", bufs=2)
                        nc.vector.tensor_mul(out=qre[:], in0=pq[:], in1=cs_sb[:, tok])
                        nc.vector.tensor_mul(out=rot[:], in0=rot[:], in1=sn_sb[:, tok])
                        nc.vector.tensor_add(out=qre[:], in0=qre[:], in1=rot[:])
                        nc.vector.tensor_copy(qf8[:, 0, 1, :], qre[:])
                        nc.scalar.activation(
                            qf8[:, 1, 1, :], qre[:],
                            mybir.ActivationFunctionType.Copy,
                        )
                return qf8

            def stage_a2(b, qc, kv, local=None):
                """post-AG (or from local replication): rms norm, kv
                up-projection, k rope; k stored fp8 in score layout."""
                kn8, vnat = kv
                src = b * 4 + qc
                tok = slice(qc * BLK, (qc + 1) * BLK)

                ssq = ps.tile([1, BLK], f32, tag="xps", bufs=1, name="ssq")
                pkv = [
                    ps.tile([128, BLK], f32, tag="proj", bufs=4, name="pkvp")
                    for _ in range(4)
                ]
                for kc in range(4):
                    if local is None:
                        latk = work.tile([128, BLK], bf16, tag="latk", bufs=4)
                        nc.scalar.dma_start(
                            latk[:],
                            ag_out[src * KVC + kc * 128 : src * KVC + (kc + 1) * 128, :],
                        )
                    else:
                        latk = local[0][kc]
                    sqc = work.tile([128, BLK], bf16, tag="sqc", bufs=2)
                    nc.scalar.activation(sqc[:], latk[:], SQUARE)
                    nc.tensor.matmul(
                        ssq[:], ones_b[:], sqc[:], start=(kc == 0), stop=(kc == 3)
                    )
                    for mc in range(4):  # [h0 nope, h0 v, h1 nope, h1 v]
                        nc.tensor.matmul(
                            pkv[mc][:],
                            wb_sb[:, kc, mc * 128 : (mc + 1) * 128],
                            latk[:],
                            start=(kc == 0),
                            stop=(kc == 3),
                        )
                if local is None:
                    kraw = work.tile([ROPE, BLK], bf16, tag="kraw", bufs=2)
                    nc.scalar.dma_start(
                        kraw[:], ag_out[src * KVC + KVR : (src + 1) * KVC, :]
                    )
                else:
                    kraw = local[1]

                # rms scale: 1/sqrt(ssq/512+eps) = exp(-0.5*ln(.))
                lnrow = work.tile([1, BLK], f32, tag="lnrow", bufs=1)
                nc.scalar.activation(lnrow[:], ssq[:], LN, bias=eps[:], scale=1.0 / KVR)
                invrow = work.tile([1, BLK], bf16, tag="invrow", bufs=1)
                nc.scalar.activation(invrow[:], lnrow[:], EXP, scale=-0.5)
                invbc_ps = ps.tile([128, BLK], f32, tag="xps", bufs=1, name="invbc_ps")
                nc.tensor.matmul(invbc_ps[:], onesrow_b[:], invrow[:])
                invbc = work.tile([128, BLK], bf16, tag="invbc", bufs=2)
                nc.vector.tensor_copy(invbc[:], invbc_ps[:])

                # k rope from gathered raw rows -> both heads' fp8 tiles
                rot = work.tile([128, BLK], bf16, tag="rot", bufs=2)
                kr = work.tile([ROPE, BLK], bf16, tag="krw", bufs=2)
                nc.vector.tensor_scalar_mul(rot[0:32, :], kraw[32:64, :], -1.0)
                nc.vector.tensor_copy(rot[32:64, :], kraw[0:32, :])
                nc.vector.tensor_mul(out=kr[:], in0=kraw[:], in1=cs_sb[0:64, tok])
                nc.vector.tensor_mul(
                    out=rot[0:64, :], in0=rot[0:64, :], in1=sn_sb[0:64, tok]
                )
                nc.vector.tensor_add(out=kr[:], in0=kr[:], in1=rot[0:64, :])
                kt4 = slice(qc * 4, qc * 4 + 4)
                nc.vector.tensor_copy(kn8[0][0:64, kt4, 1, :], kr[:])
                nc.scalar.activation(
                    kn8[1][64:128, kt4, 1, :], kr[:],
                    mybir.ActivationFunctionType.Copy,
                )

                # normalize + store k_nope (fp8) and v (bf16, natural layout)
                for mc in range(4):
                    h = mc // 2
                    if mc % 2 == 0:
                        nc.vector.tensor_mul(
                            out=kn8[h][:, kt4, 0, :], in0=pkv[mc][:], in1=invbc[:]
                        )
                    else:
                        vuT = work.tile([128, BLK], bf16, tag="vuT", bufs=2)
                        nc.vector.tensor_mul(out=vuT[:], in0=pkv[mc][:], in1=invbc[:])
                        nc.sync.dma_start_transpose(
                            vnat[h][:, qc * 4 : qc * 4 + 4, :], vuT[:]
                        )

            def stage_bh(b, qc, h, qf8, kv):
                """causal attention for one q-chunk, one head.

                One fp8 DoubleRow matmul per score tile (k_nope + padded
                k_rope contract together). exp/mask/acc/yacc trim to the
                valid diagonal column ranges. Lookahead-2; denominator on
                Vector in f32r."""
                kn8, vnat = kv
                n_kt = 4 * (qc + 1)
                yacc = ps.tile([VD, BLK], f32, tag="yacc", bufs=1)
                acc = work.tile([128, BLK], f32r, tag="acc", bufs=2)
                accg = work.tile([128, BLK], f32r, tag="accg", bufs=2)
                pend = []
                for kt in range(n_kt):
                    j = kt - 4 * qc
                    c0 = max(j, 0) * 128  # first valid q column
                    qs = slice(c0, BLK)
                    st = ps.tile([128, BLK], f32, tag="st", bufs=2)
                    nc.tensor.matmul(
                        st[:], kn8[h][:, kt, :, :], qf8[:, h, :, :],
                        start=True, stop=True, perf_mode=DR,
                    )
                    if len(pend) == 2:
                        pe_est, pk, pqs = pend.pop(0)
                        nc.tensor.matmul(
                            yacc[:, pqs], vnat[h][:, pk, :], pe_est[:, pqs],
                            start=(pk == 0), stop=False,
                        )
                    est = work.tile([128, BLK], bf16, tag="est", bufs=5)
                    nc.scalar.activation(est[:, qs], st[:, qs], EXP, scale=1.0 / S)
                    if j >= 0:
                        nc.vector.tensor_mul(
                            out=est[:, qs], in0=est[:, qs],
                            in1=msk_sb[:, j * BLK + c0 : (j + 1) * BLK],
                        )
                    if kt % 3 == 2:  # every third tile accumulates on gpsimd
                        if kt == 2:
                            nc.gpsimd.tensor_copy(accg[:, qs], est[:, qs])
                            if c0:
                                nc.gpsimd.memset(accg[:, :c0].bitcast(f32), 0.0)
                        else:
                            nc.gpsimd.tensor_add(
                                out=accg[:, qs], in0=accg[:, qs], in1=est[:, qs]
                            )
                    elif kt == 0:
                        nc.vector.tensor_copy(acc[:], est[:])
                    else:
                        nc.vector.tensor_add(
                            out=acc[:, qs], in0=acc[:, qs], in1=est[:, qs]
                        )
                    pend.append((est, kt, qs))
                for pe_est, pk, pqs in pend:
                    nc.tensor.matmul(
                        yacc[:, pqs], vnat[h][:, pk, :], pe_est[:, pqs],
                        start=(pk == 0), stop=(pk == n_kt - 1),
                    )

                sums = ps.tile([1, BLK], f32, tag="xps", bufs=1, name="sums")
                nc.tensor.matmul(sums[:], ones_r[:], acc[:], start=True, stop=False)
                nc.tensor.matmul(sums[:], ones_r[:], accg[:], start=False, stop=True)
                lnr = work.tile([1, BLK], f32, tag="lnrow", bufs=1)
                nc.scalar.activation(lnr[:], sums[:], LN)
                sinvrow = work.tile([1, BLK], bf16, tag="invrow", bufs=1)
                nc.scalar.activation(sinvrow[:], lnr[:], EXP, scale=-1.0)
                sbc_ps = ps.tile([128, BLK], f32, tag="xps", bufs=1, name="sbc_ps")
                nc.tensor.matmul(sbc_ps[:], onesrow_b[:], sinvrow[:])
                sinv = work.tile([128, BLK], bf16, tag="sinv", bufs=1)
                nc.vector.tensor_copy(sinv[:], sbc_ps[:])
                ysb = work.tile([VD, BLK], bf16, tag="ysb", bufs=1)
                nc.vector.tensor_mul(out=ysb[:], in0=yacc[:], in1=sinv[:])
                for jj in range(2):
                    if b == 0:
                        nc.sync.dma_start(
                            y_in[0][qc * 2 + jj, h * VD : (h + 1) * VD, :],
                            ysb[:, jj * 256 : (jj + 1) * 256],
                        )
                    else:
                        nc.sync.dma_start(
                            y_in1h[h][qc * 2 + jj, :, :],
                            ysb[:, jj * 256 : (jj + 1) * 256],
                        )

            def emit_a2a(b):
                nc.gpsimd.collective_compute(
                    "AllToAll",
                    mybir.AluOpType.bypass,
                    replica_groups=RG,
                    ins=[y_in[b].opt()],
                    outs=[y_out[b].opt()],
                )

            def emit_a2a1h(h):
                nc.gpsimd.collective_compute(
                    "AllToAll",
                    mybir.AluOpType.bypass,
                    replica_groups=RG,
                    ins=[y_in1h[h].opt()],
                    outs=[y_out1h[h].opt()],
                )

            a2a_sb = {}

            def wo_load(b):
                a2a = wop.tile([128, 16, 256], bf16, tag="a2a", bufs=1, name=f"a2a{b}")
                if b == 0:
                    nc.sync.dma_start(
                        a2a[:], y_out[0][:].rearrange("c (h p) t -> p (c h) t", p=128)
                    )
                else:
                    for h in range(2):
                        nc.sync.dma_start(
                            a2a[:, h * 8 : h * 8 + 8, :],
                            y_out1h[h][:].rearrange("c p t -> p c t"),
                        )
                a2a_sb[b] = a2a

            def wo_chains(b, chains):
                """wo projection chains for this batch's gathered token slice."""
                a2a = a2a_sb[b]
                # batch 1's staging holds [h0 cores 0..7 | h1 cores 0..7]; wo
                # rows are (core, h) interleaved
                order = (
                    list(range(16))
                    if b == 0
                    else [h * 8 + c for h in range(2) for c in range(8)]
                )
                wokc = (
                    list(range(16))
                    if b == 0
                    else [c * 2 + h for h in range(2) for c in range(8)]
                )
                for n, tt in chains:
                    pout = ps.tile([128, 512], f32, tag="proj", bufs=4, name="pout")
                    for i in range(16):
                        nc.tensor.matmul(
                            pout[:],
                            a2a[:, order[i], tt * 128 : (tt + 1) * 128],
                            wo_box["wo"][:, wokc[i], n * 512 : (n + 1) * 512],
                            start=(i == 0),
                            stop=(i == 15),
                        )
                    osb = wop.tile([128, 512], bf16, tag="osb", bufs=2)
                    nc.scalar.activation(
                        osb[:], pout[:], mybir.ActivationFunctionType.Copy
                    )
                    nc.sync.dma_start(
                        out_d[
                            b, tt * 128 : (tt + 1) * 128, n * 512 : (n + 1) * 512
                        ],
                        osb[:],
                    )

            ALLC = [(n, tt) for n in range(4) for tt in range(2)]

            # ---- schedule (Tile reorders by dependencies; emission order
            # mostly matters for same-engine queue order) ----
            kv0 = alloc_kv(0)
            kv1 = alloc_kv(1)
            kvc_block()
            load_msk()
            q00 = stage_a1(0, 0)
            q01 = stage_a1(0, 1)
            loc0 = rep_kvc(0)
            stage_a2(0, 0, kv0, local=loc0)
            loc1 = rep_kvc(1)
            stage_a2(0, 1, kv0, local=loc1)
            stage_bh(0, 0, 0, q00, kv0)
            q02 = stage_a1(0, 2)
            stage_bh(0, 0, 1, q00, kv0)
            q03 = stage_a1(0, 3)
            stage_bh(0, 1, 0, q01, kv0)
            q10 = stage_a1(1, 0)
            stage_bh(0, 1, 1, q01, kv0)
            q11 = stage_a1(1, 1)
            q12 = stage_a1(1, 2)
            load_wo()
            q13 = stage_a1(1, 3)
            stage_a2(0, 2, kv0)
            stage_bh(0, 2, 0, q02, kv0)
            stage_bh(0, 2, 1, q02, kv0)
            stage_a2(0, 3, kv0)
            stage_bh(0, 3, 0, q03, kv0)
            stage_a2(1, 0, kv1)
            stage_bh(0, 3, 1, q03, kv0)
            emit_a2a(0)
            stage_a2(1, 1, kv1)
            stage_bh(1, 0, 0, q10, kv1)
            stage_bh(1, 0, 1, q10, kv1)
            stage_a2(1, 2, kv1)
            wo_load(0)
            wo_chains(0, ALLC[0:2])
            stage_bh(1, 1, 0, q11, kv1)
            stage_bh(1, 1, 1, q11, kv1)
            stage_a2(1, 3, kv1)
            wo_chains(0, ALLC[2:4])
            stage_bh(1, 2, 0, q12, kv1)
            stage_bh(1, 2, 1, q12, kv1)
            stage_bh(1, 3, 0, q13, kv1)
            emit_a2a1h(0)
            stage_bh(1, 3, 1, q13, kv1)
            emit_a2a1h(1)
            wo_chains(0, ALLC[4:8])
            wo_load(1)
            wo_chains(1, ALLC)

    nc.compile()
    return nc


def host_prep(x, wq, wkv_a, wkv_b, wo, kv_norm_w):
    bf = ml_dtypes.bfloat16
    f8 = ml_dtypes.float8_e4m3fn
    scale = np.float32(QKD ** -0.5)
    inv = (1.0 / (10000.0 ** (np.arange(0, ROPE, 2, dtype=np.float32) / ROPE))).astype(
        np.float32
    )
    f = np.outer(np.arange(T, dtype=np.float32), inv)
    cos32 = np.cos(f).T.astype(np.float32)
    sin32 = np.sin(f).T.astype(np.float32)
    cos128 = np.ascontiguousarray(np.concatenate([cos32] * 4, 0)).astype(bf)
    sin128 = np.ascontiguousarray(np.concatenate([sin32] * 4, 0)).astype(bf)
    wkv_bw = (wkv_b * kv_norm_w[:, None]).astype(np.float32)
    xtf = np.ascontiguousarray(x.reshape(B * T, D).T)
    xt8 = xtf.astype(f8)
    wa = np.ascontiguousarray(wkv_a).astype(bf)
    wo_c = np.ascontiguousarray(wo).astype(bf)
    wq_r = wq.reshape(D, H, QKD)

    kk = np.arange(128)[:, None]
    qq = np.arange(BLK)[None, :]
    msk = np.concatenate(
        [(qq >= kk + j * 128).astype(np.float32) for j in range(4)], axis=1
    ).astype(bf)

    in_maps = []
    for c in range(NCORES):
        h0 = HPC * c
        w1f = np.concatenate(
            [
                wq_r[:, h0, :NOPE],
                wq_r[:, h0 + 1, :NOPE],
                wq_r[:, h0, NOPE:],
                wq_r[:, h0 + 1, NOPE:],
            ],
            axis=1,
        ) * (scale * S)
        # permute to rows (kp, p), cols (mt, two, mw): each DoubleRow
        # stationary slice [p, two, mw] must be contiguous in SBUF
        w1p = w1f.reshape(8, 2, 128, 3, 128).transpose(0, 2, 3, 1, 4)
        w1 = np.ascontiguousarray(w1p.reshape(1024, 768)).astype(f8)
        xslice = np.ascontiguousarray(xtf[:, c * BLK : (c + 1) * BLK]).astype(bf)
        xs01 = np.ascontiguousarray(xtf[:, : 2 * BLK]).astype(bf)
        wb = np.ascontiguousarray(
            wkv_bw[:, h0 * (NOPE + VD) : (h0 + 2) * (NOPE + VD)]
        ).astype(bf)
        in_maps.append(
            {
                "xt8": xt8,
                "xs": xslice,
                "xs01": xs01,
                "w1": w1,
                "wa": wa,
                "wb": wb,
                "wo": wo_c,
                "cos": cos128,
                "sin": sin128,
                "msk": msk,
            }
        )
    return in_maps


_NC = None


def kernel(x, wq, wkv_a, wkv_b, wo, kv_norm_w, _trace=False):
    global _NC
    if _NC is None:
        _NC = build_program()
    in_maps = host_prep(
        np.asarray(x, np.float32),
        np.asarray(wq, np.float32),
        np.asarray(wkv_a, np.float32),
        np.asarray(wkv_b, np.float32),
        np.asarray(wo, np.float32),
        np.asarray(kv_norm_w, np.float32),
    )
    res = run_bass_kernel_spmd(_NC, in_maps, list(range(NCORES)), trace=_trace)
    out = np.empty((B, T, D), np.float32)
    cw = T // NCORES
    for c in range(NCORES):
        oc = res.results[c]["out"].astype(np.float32)  # (B, 256, D)
        for b in range(B):
            out[b, c * cw : (c + 1) * cw, :] = oc[b]
    kernel.last_results = res
    return out
